# revision 1
# baseline (speedup 1.0000x reference)
"""Trainium2 Bass kernel for nn_EnhancedFractionalPINO.

Pipeline (per core, batch-parallel over 8 NeuronCores, 32 batches/core):
  1. f = Re(fft2(x)) per 64x64 image via cosine/sine DFT matmuls:
     m1: per image, lhsT = image, rhs = [C | S] -> [x^T C | x^T S];
     m2: per 8-image group, two const-stationary matmuls with strided rhs
     -> A^T = C x^T C - S x^T S for all 8 images in one psum tile.
  2. GL fractional derivative = truncated causal conv (KTAPS taps) over the
     globally-flattened signal, as Toeplitz-block matmuls (halo image passed
     from the previous core's batch range; zeros for core 0). The h^-alpha
     scale is folded into Ws1 so everything stays in fp16 range.
  3. spectral_operator + neural_operator MLPs as fp16 PE matmuls with a
     positive rescaling chain (LAM_*) keeping activations in fp16 range;
     activations-stationary, PE transposes between layers.
  4. out = Re(ifft2(proc)) via the same DFT-matmul machinery (scales folded
     into the second-stage constants).

Weights are replicated across cores; activations stay SBUF-resident.
"""

import numpy as np

import concourse.bass as bass
import concourse.mybir as mybir
import concourse.tile as tile
from concourse import bacc
from concourse.bass_utils import run_bass_kernel_spmd

F32 = mybir.dt.float32
F16 = mybir.dt.float16
AF = mybir.ActivationFunctionType

B, C, H, W = 256, 3, 64, 64
MODES = C * H * W              # 12288
ALPHA = 0.5
NTOT = B * MODES               # 3145728 flattened samples
NCORE = 8
BS = B // NCORE                # 32 batches per core
NIMG = BS * C                  # 96 images per core
NSLOT = NIMG + 2               # halo + 96 images + zero pad
KTAPS = 512                    # truncated GL taps (4 chunks of 128)
NCH = BS * MODES // 128        # 3072 output chunks per core
NBLK = NCH // 512              # 6 conv blocks of 512 chunks

# fp16 activation rescaling chain (see mirror3 validation)
LAM_H, LAM_S, LAM_1, LAM_2, LAM_P = 16.0, 8.0, 4.0, 4.0, 4.0


# ---------------------------------------------------------------- host consts
def _host_constants():
    jk = np.outer(np.arange(64), np.arange(64)).astype(np.float64)
    Cm = np.cos(2 * np.pi * jk / 64)
    Sm = np.sin(2 * np.pi * jk / 64)

    j = np.arange(1, KTAPS, dtype=np.float64)
    w = np.concatenate([[1.0], np.cumprod((j - 1.0 - ALPHA) / j)])

    # Tst[d][t, tau] = w[128*d + tau - t]  (lhsT layout of the Toeplitz blocks)
    idx = 128 * np.arange(4)[:, None, None] \
        + np.arange(128)[None, None, :] - np.arange(128)[None, :, None]
    Tst = np.where((idx >= 0) & (idx < KTAPS), w[np.clip(idx, 0, KTAPS - 1)], 0.0)

    f16 = lambda a: np.ascontiguousarray(a, dtype=np.float16)
    return {
        "cswi": f16(np.concatenate([Cm, Sm], axis=1)),     # [64, 128]
        "cmf": f16(Cm),                                    # [64, 64]
        "msf": f16(-Sm),
        "cmi": f16(Cm * (LAM_P / 4096.0)),
        "smi": f16(-Sm * (LAM_P / 4096.0)),
        "tst": f16(Tst),
        "idn32": f16(np.eye(32)),
        "ones1": f16(np.ones((1, 32))),
    }


def _prep_weights(Ws1, bs1, Ws2, bs2, Wn1, bn1, Wn2, bn2, Wn3, bn3):
    s = float(np.float64(1.0 / (NTOT - 1)) ** (-ALPHA))
    f16 = lambda a: np.ascontiguousarray(a, dtype=np.float16)
    W1 = (Ws1.astype(np.float64) * (s / LAM_H)).astype(np.float32)
    W2 = Ws2 * np.float32(LAM_H / LAM_S)
    W3 = Wn1 * np.float32(LAM_S / LAM_1)
    W4 = Wn2 * np.float32(LAM_1 / LAM_2)
    W5 = Wn3 * np.float32(LAM_2 / LAM_P)
    return {
        "w1t": f16(W1.reshape(24, 4, 128, 512).transpose(0, 2, 1, 3)),
        "w2r": f16(W2.reshape(4, 128, 12, 1024).transpose(2, 1, 0, 3)),
        "w3t": f16(W3.reshape(24, 4, 128, 512).transpose(0, 2, 1, 3)),
        "w4t": f16(W4.reshape(4, 128, 4, 128).transpose(2, 1, 0, 3)
                   .reshape(4, 128, 512)),
        "w5r": f16(W5.reshape(4, 128, 12, 1024).transpose(2, 1, 0, 3)),
        "b1r": f16((bs1 / LAM_H).reshape(1, 512)),
        "b2r": f16((bs2 / LAM_S).reshape(1, MODES)),
        "b3r": f16((bn1 / LAM_1).reshape(1, 512)),
        "b4t": np.ascontiguousarray((bn2 / LAM_2).reshape(4, 128).T,
                                    dtype=np.float32),     # [128, 4]
        "b5r": f16((bn3 / LAM_P).reshape(1, MODES)),
    }


# ---------------------------------------------------------------- bass module
_NC_CACHE = None


def _build_nc():
    nc = bacc.Bacc("TRN2", target_bir_lowering=False, debug=False,
                   num_devices=NCORE)

    def din(name, shape, dt=F16):
        return nc.dram_tensor(name, shape, dt, kind="ExternalInput")

    d_x = din("ximgs", (NSLOT, 64, 64))
    d_cswi = din("cswi", (64, 128))
    d_cmf = din("cmf", (64, 64))
    d_msf = din("msf", (64, 64))
    d_cmi = din("cmi", (64, 64))
    d_smi = din("smi", (64, 64))
    d_tst = din("tst", (4, 128, 128))
    d_idn = din("idn32", (32, 32))
    d_ones = din("ones1", (1, 32))
    d_w1 = din("w1t", (24, 128, 4, 512))
    d_w2 = din("w2r", (12, 128, 4, 1024))
    d_w3 = din("w3t", (24, 128, 4, 512))
    d_w4 = din("w4t", (4, 128, 512))
    d_w5 = din("w5r", (12, 128, 4, 1024))
    d_b1 = din("b1r", (1, 512))
    d_b2 = din("b2r", (1, MODES))
    d_b3 = din("b3r", (1, 512))
    d_b4 = nc.dram_tensor("b4t", (128, 4), F32, kind="ExternalInput")
    d_b5 = din("b5r", (1, MODES))
    d_out = nc.dram_tensor("out", (BS, C, 64, 64), F32, kind="ExternalOutput")

    with tile.TileContext(nc) as tc:
        with tc.tile_pool(name="cpool", bufs=1) as cpool, \
             tc.tile_pool(name="bigpool", bufs=1) as bigpool:
            # ---- constants into SBUF
            cswi = cpool.tile([64, 128], F16, tag="cswi")
            cmf = cpool.tile([64, 64], F16, tag="cmf")
            msf = cpool.tile([64, 64], F16, tag="msf")
            cmi = cpool.tile([64, 64], F16, tag="cmi")
            smi = cpool.tile([64, 64], F16, tag="smi")
            tsb = cpool.tile([128, 4, 128], F16, tag="tsb")
            idn = cpool.tile([32, 32], F16, tag="idn")
            ones1 = cpool.tile([1, 32], F16, tag="ones1")
            b1s = cpool.tile([1, 512], F16, tag="b1s")
            b3s = cpool.tile([1, 512], F16, tag="b3s")
            b4s = cpool.tile([128, 4], F32, tag="b4s")
            bbig = cpool.tile([1, MODES], F16, tag="bbig")  # b2 then b5
            for t, d in ((cswi, d_cswi), (cmf, d_cmf), (msf, d_msf),
                         (cmi, d_cmi), (smi, d_smi), (idn, d_idn),
                         (ones1, d_ones), (b1s, d_b1), (b3s, d_b3),
                         (b4s, d_b4)):
                nc.sync.dma_start(t[:], d[:])
            nc.sync.dma_start(tsb[:], d_tst.rearrange("d p k -> p d k"))

            # ---- persistent activation tiles
            fbuf = bigpool.tile([128, 4 + NCH + 64], F16, tag="fbuf")
            frlin = bigpool.tile([128, NCH], F16, tag="frlin")
            specT = bigpool.tile([128, 96, BS], F16, tag="specT")
            procTs = [bigpool.tile([64, 64, BS], F16, tag=f"procT{i}",
                                   name=f"procT{i}") for i in range(C)]
            hT = bigpool.tile([128, 4, BS], F16, tag="hT")
            h1T = bigpool.tile([128, 4, BS], F16, tag="h1T")
            h2T = bigpool.tile([128, 4, BS], F16, tag="h2T")
            h_sb = bigpool.tile([32, 512], F16, tag="h_sb")
            h1_sb = bigpool.tile([32, 512], F16, tag="h1_sb")

            # ========== phase 1: fft2 (per-image m1, 8-wide m2) =============
            with tc.tile_pool(name="xpool", bufs=1) as xpool, \
                 tc.tile_pool(name="gpool", bufs=6) as gpool, \
                 tc.tile_pool(name="ps1p", bufs=4, space="PSUM") as ps1p, \
                 tc.tile_pool(name="ps2p", bufs=3, space="PSUM") as ps2p:
                xall = xpool.tile([64, NSLOT, 64], F16, tag="xall")
                for ch in range(4):
                    q0 = (NSLOT * ch) // 4
                    q1 = (NSLOT * (ch + 1)) // 4
                    nc.sync.dma_start(
                        xall[:, q0:q1, :],
                        d_x[q0:q1].rearrange("q p k -> p q k"))
                for grp in range(25):
                    n = 4 if grp < 24 else 2
                    psA = ps1p.tile([64, 512], F32, tag="psA")
                    for t in range(n):
                        i = grp * 4 + t
                        nc.tensor.matmul(psA[:, t * 128:(t + 1) * 128],
                                         xall[:, i, :], cswi[:],
                                         start=True, stop=True)
                    g1w = gpool.tile([64, 4, 128], F16, tag="g1w")
                    g1f = g1w[:, 0:n, :].rearrange("p a k -> p (a k)")
                    if grp % 2 == 0:
                        nc.scalar.copy(g1f, psA[:, 0:n * 128])
                    else:
                        nc.vector.tensor_copy(g1f, psA[:, 0:n * 128])
                    ps2 = ps2p.tile([64, 256], F32, tag="ps2")
                    nc.tensor.matmul(ps2[:, 0:n * 64], cmf[:],
                                     g1w[:, 0:n, 0:64], start=True, stop=False)
                    nc.tensor.matmul(ps2[:, 0:n * 64], msf[:],
                                     g1w[:, 0:n, 64:128], start=False, stop=True)
                    p2v = ps2.rearrange("p (k two) -> p k two", two=2)
                    if grp == 0:
                        # halo image: last 4 chunk-cols; imgs 1..3 -> cols 4:100
                        nc.vector.tensor_copy(fbuf[0:64, 0:4], p2v[:, 28:32, 0])
                        nc.vector.tensor_copy(fbuf[64:128, 0:4], p2v[:, 28:32, 1])
                        nc.vector.tensor_copy(fbuf[0:64, 4:100], p2v[:, 32:128, 0])
                        nc.vector.tensor_copy(fbuf[64:128, 4:100],
                                              p2v[:, 32:128, 1])
                    else:
                        base = 4 + (grp * 4 - 1) * 32
                        nc.vector.tensor_copy(fbuf[0:64, base:base + n * 32],
                                              p2v[:, 0:n * 32, 0])
                        nc.vector.tensor_copy(fbuf[64:128, base:base + n * 32],
                                              p2v[:, 0:n * 32, 1])

            # ================= phase 2: conv ================================
            with tc.tile_pool(name="pscv2", bufs=1, space="PSUM") as pscv2:
                psc = [pscv2.tile([128, 512], F32, tag=f"psc{i}",
                                  name=f"psc{i}") for i in range(NBLK)]
                for d in range(4):
                    for blk in range(NBLK):
                        o = 4 + blk * 512 - d
                        nc.tensor.matmul(psc[blk][:], tsb[:, d, :],
                                         fbuf[:, o:o + 512],
                                         start=(d == 0), stop=(d == 3))
                for blk in range(NBLK):
                    nc.vector.tensor_copy(frlin[:, blk * 512:(blk + 1) * 512],
                                          psc[blk][:])

            frl3 = frlin.rearrange("p (b k) -> p b k", b=BS)

            # ======= L1 / L3: acts-stationary 12288->512 + relu + transpose =
            def big_layer(src_blk, d_w, bias_row, out_sb, outT, dma_eng):
                with tc.tile_pool(name="wp", bufs=14) as wp, \
                     tc.tile_pool(name="psm", bufs=1, space="PSUM") as psm, \
                     tc.tile_pool(name="pst", bufs=1, space="PSUM") as pst:
                    acc = psm.tile([32, 512], F32, tag="acc")
                    for K4 in range(24):
                        wt = wp.tile([128, 4, 512], F16, tag="wt")
                        dma_eng.dma_start(wt[:], d_w[K4])
                        for j in range(4):
                            nc.tensor.matmul(acc[:], src_blk(4 * K4 + j),
                                             wt[:, j, :],
                                             start=(K4 == 0 and j == 0),
                                             stop=False)
                    nc.tensor.matmul(acc[:], ones1[:], bias_row[:],
                                     start=False, stop=True)
                    nc.scalar.activation(out_sb[:], acc[:], AF.Relu)
                    pt = pst.tile([128, 128], F16, tag="pt")
                    for fb in range(4):
                        nc.tensor.transpose(pt[:, fb * 32:(fb + 1) * 32],
                                            out_sb[:, fb * 128:(fb + 1) * 128],
                                            idn[:])
                    nc.vector.tensor_copy(
                        outT[:], pt.rearrange("p (f b) -> p f b", f=4))

            big_layer(lambda K: frl3[:, :, K], d_w1, b1s, h_sb, hT, nc.sync)

            # ======= L2 + L3, emission-interleaved ==========================
            # L3's k-block K only needs L2's chunk K//4, and PSUM accumulation
            # is order-independent, so L3's matmuls ride along the L2 loop.
            nc.sync.dma_start(bbig[:], d_b2[:])
            with tc.tile_pool(name="wp2", bufs=4) as wp2, \
                 tc.tile_pool(name="wp3", bufs=3) as wp3, \
                 tc.tile_pool(name="sp2", bufs=3) as sp2, \
                 tc.tile_pool(name="ps2m", bufs=3, space="PSUM") as ps2m, \
                 tc.tile_pool(name="pst2", bufs=3, space="PSUM") as pst2, \
                 tc.tile_pool(name="psm3", bufs=1, space="PSUM") as psm3:
                acc3 = psm3.tile([32, 512], F32, tag="acc3")
                for mc2 in range(12):
                    wt = wp2.tile([128, 4, 1024], F16, tag="w2")
                    nc.sync.dma_start(wt[:], d_w2[mc2])
                    for half in range(2):
                        mc = 2 * mc2 + half
                        acc = ps2m.tile([32, 512], F32, tag="acc2")
                        for fb in range(4):
                            nc.tensor.matmul(
                                acc[:], hT[:, fb, :],
                                wt[:, fb, half * 512:(half + 1) * 512],
                                start=(fb == 0), stop=False)
                        nc.tensor.matmul(acc[:], ones1[:],
                                         bbig[0:1, mc * 512:(mc + 1) * 512],
                                         start=False, stop=True)
                        sb = sp2.tile([32, 512], F16, tag="sb2")
                        if half == 0:
                            nc.scalar.copy(sb[:], acc[:])
                        else:
                            nc.vector.tensor_copy(sb[:], acc[:])
                        pt = pst2.tile([128, 128], F16, tag="pt2")
                        for fb in range(4):
                            nc.tensor.transpose(pt[:, fb * 32:(fb + 1) * 32],
                                                sb[:, fb * 128:(fb + 1) * 128],
                                                idn[:])
                        nc.vector.tensor_copy(
                            specT[:, mc * 4:(mc + 1) * 4, :],
                            pt.rearrange("p (f b) -> p f b", f=4))
                    # L3 portion: k-blocks for the two chunks just produced
                    wt3 = wp3.tile([128, 4, 512], F16, tag="wt3")
                    nc.scalar.dma_start(wt3[:], d_w3[2 * mc2])
                    wt3b = wp3.tile([128, 4, 512], F16, tag="wt3b")
                    nc.scalar.dma_start(wt3b[:], d_w3[2 * mc2 + 1])
                    for K4, w3t in ((2 * mc2, wt3), (2 * mc2 + 1, wt3b)):
                        for j in range(4):
                            nc.tensor.matmul(acc3[:],
                                             specT[:, 4 * K4 + j, :],
                                             w3t[:, j, :],
                                             start=(mc2 == 0 and K4 == 0
                                                    and j == 0),
                                             stop=False)
                nc.tensor.matmul(acc3[:], ones1[:], b3s[:],
                                 start=False, stop=True)
                nc.scalar.activation(h1_sb[:], acc3[:], AF.Relu)
                with tc.tile_pool(name="pst3", bufs=1, space="PSUM") as pst3:
                    pt = pst3.tile([128, 128], F16, tag="pt3")
                    for fb in range(4):
                        nc.tensor.transpose(pt[:, fb * 32:(fb + 1) * 32],
                                            h1_sb[:, fb * 128:(fb + 1) * 128],
                                            idn[:])
                    nc.vector.tensor_copy(
                        h1T[:], pt.rearrange("p (f b) -> p f b", f=4))

            # ======= L4: weights-stationary 512->512 + relu =================
            with tc.tile_pool(name="wp4", bufs=1) as wp4, \
                 tc.tile_pool(name="ps4m", bufs=2, space="PSUM") as ps4m:
                w4 = wp4.tile([128, 4, 512], F16, tag="w4")
                nc.gpsimd.dma_start(w4[:], d_w4.rearrange("a p k -> p a k"))
                for f2b in range(4):
                    acc = ps4m.tile([128, 32], F32, tag="acc4")
                    for fb in range(4):
                        nc.tensor.matmul(acc[:],
                                         w4[:, f2b, fb * 128:(fb + 1) * 128],
                                         h1T[:, fb, :],
                                         start=(fb == 0), stop=(fb == 3))
                    nc.scalar.activation(h2T[:, f2b, :], acc[:], AF.Relu,
                                         bias=b4s[:, f2b:f2b + 1])

            # ======= L5 + ifft2, emission-interleaved by channel ============
            nc.sync.dma_start(bbig[:], d_b5[:])
            with tc.tile_pool(name="wp5", bufs=5) as wp5, \
                 tc.tile_pool(name="sp5", bufs=3) as sp5, \
                 tc.tile_pool(name="opool", bufs=1) as opool, \
                 tc.tile_pool(name="gpi", bufs=2) as gpi, \
                 tc.tile_pool(name="ps5m", bufs=2, space="PSUM") as ps5m, \
                 tc.tile_pool(name="pst5", bufs=2, space="PSUM") as pst5, \
                 tc.tile_pool(name="ps1i", bufs=2, space="PSUM") as ps1i, \
                 tc.tile_pool(name="ps2i", bufs=2, space="PSUM") as ps2i:
                oall = opool.tile([64, NIMG * 64], F32, tag="oall")
                oal3 = oall.rearrange("u (b c v) -> u b c v", b=BS, c=C)

                def ifft2_channel(c):
                    for bg in range(BS // 4):
                        psA = ps1i.tile([64, 512], F32, tag="psAi",
                                        name="psAi")
                        for t in range(4):
                            b = bg * 4 + t
                            nc.tensor.matmul(psA[:, t * 128:(t + 1) * 128],
                                             procTs[c][:, :, b],
                                             cswi[:], start=True, stop=True)
                        g1w = gpi.tile([64, 4, 128], F16, tag="g1i",
                                       name="g1i")
                        if bg % 2 == 0:
                            nc.scalar.copy(g1w.rearrange("p a k -> p (a k)"),
                                           psA[:])
                        else:
                            nc.vector.tensor_copy(
                                g1w.rearrange("p a k -> p (a k)"), psA[:])
                        ps2 = ps2i.tile([64, 256], F32, tag="p2i", name="p2i")
                        nc.tensor.matmul(ps2[:], cmi[:], g1w[:, :, 0:64],
                                         start=True, stop=False)
                        nc.tensor.matmul(ps2[:], smi[:], g1w[:, :, 64:128],
                                         start=False, stop=True)
                        nc.scalar.copy(
                            oal3[:, bg * 4:(bg + 1) * 4, c, :],
                            ps2.rearrange("u (b v) -> u b v", b=4))
                        if c == 2:
                            for b0 in (bg * 4, bg * 4 + 2):
                                nc.sync.dma_start(
                                    d_out[b0:b0 + 2].rearrange(
                                        "b c u v -> u b c v"),
                                    oall[:, b0 * 192:(b0 + 2) * 192].rearrange(
                                        "u (b c v) -> u b c v", b=2, c=C))

                for mc2 in range(12):
                    wt = wp5.tile([128, 4, 1024], F16, tag="w5")
                    nc.gpsimd.dma_start(wt[:], d_w5[mc2])
                    for half in range(2):
                        mc = 2 * mc2 + half
                        acc = ps5m.tile([32, 512], F32, tag="acc5")
                        for fb in range(4):
                            nc.tensor.matmul(
                                acc[:], h2T[:, fb, :],
                                wt[:, fb, half * 512:(half + 1) * 512],
                                start=(fb == 0), stop=False)
                        nc.tensor.matmul(acc[:], ones1[:],
                                         bbig[0:1, mc * 512:(mc + 1) * 512],
                                         start=False, stop=True)
                        sb = sp5.tile([32, 512], F16, tag="sb5")
                        if half == 0:
                            nc.scalar.copy(sb[:], acc[:])
                        else:
                            nc.vector.tensor_copy(sb[:], acc[:])
                        pt = pst5.tile([64, 256], F16, tag="pt5")
                        for t in range(8):
                            nc.tensor.transpose(pt[:, t * 32:(t + 1) * 32],
                                                sb[:, t * 64:(t + 1) * 64],
                                                idn[:])
                        nc.vector.tensor_copy(
                            procTs[mc // 8][:, (mc % 8) * 8:(mc % 8 + 1) * 8, :],
                            pt.rearrange("p (t b) -> p t b", t=8))
                    if mc2 in (3, 7, 11):
                        ifft2_channel(mc2 // 4)

    nc.compile()
    return nc


def _get_nc():
    global _NC_CACHE
    if _NC_CACHE is None:
        _NC_CACHE = _build_nc()
    return _NC_CACHE


def _make_in_maps(x, Ws1, bs1, Ws2, bs2, Wn1, bn1, Wn2, bn2, Wn3, bn3):
    shared = dict(_host_constants())
    shared.update(_prep_weights(Ws1, bs1, Ws2, bs2, Wn1, bn1, Wn2, bn2,
                                Wn3, bn3))
    in_maps = []
    for g in range(NCORE):
        if g == 0:
            halo = np.zeros((1, 64, 64), np.float32)
        else:
            halo = x[g * BS - 1, 2][None]
        ximgs = np.concatenate(
            [halo, x[g * BS:(g + 1) * BS].reshape(NIMG, 64, 64),
             np.zeros((1, 64, 64), np.float32)]).astype(np.float16)
        in_maps.append({"ximgs": np.ascontiguousarray(ximgs), **shared})
    return in_maps


def kernel(**inputs):
    x = np.ascontiguousarray(inputs["x"], dtype=np.float32)
    nc = _get_nc()
    in_maps = _make_in_maps(
        x, inputs["Ws1"], inputs["bs1"], inputs["Ws2"], inputs["bs2"],
        inputs["Wn1"], inputs["bn1"], inputs["Wn2"], inputs["bn2"],
        inputs["Wn3"], inputs["bn3"])
    res = run_bass_kernel_spmd(nc, in_maps, list(range(NCORE)))
    out = np.empty((B, C, H, W), np.float32)
    for g in range(NCORE):
        out[g * BS:(g + 1) * BS] = res.results[g]["out"]
    return out



# revision 2
# speedup vs baseline: 2.2727x; 2.2727x over previous
"""Trainium2 Bass kernel for nn_EnhancedFractionalPINO.

Math folding (all precomputed on host, per call):
  reference out = iDFT( relu(relu(relu(GLconv(DFT(x))@Ws1+b1) @ (Ws2@Wn1)
                  + (bs2@Wn1+bn1)) @ Wn2 + bn2) @ Wn3 + bn3 )
  - Ws2@Wn1 folds to a single 512x512 matrix U (no relu between the two
    12288-wide matmuls in the reference), eliminating both of them.
  - The GL fractional conv (lower-triangular Toeplitz T0 within a batch row
    plus a 512-sample halo from the previous batch) and the forward 2-D DFT
    fold into Ws1:  V0_pix = D^T T0^T Ws1 acts on raw pixels;  a 512x512
    V1h acts on the last 512 DFT values of the previous batch's channel-2
    image (computed on host via fft2).  Full in-batch GL taps -> more
    accurate than any on-device truncation.
  - The inverse 2-D DFT folds into Wn3: W5f = Wn3 o blockdiag(Re(iDFT));
    the device's final matmul emits the finished result.

Per core (batch-parallel, 32 batches/core): a 4-layer MLP
  h   = relu([halo | x_pixels] @ [V1h; V0_pix] + b1)      (K=12800 streamed)
  h1  = relu(h @ U + cU);  h2 = relu(h1 @ W4 + b4)        (weights resident)
  out = h2 @ W5f + b5f                                    (N=12288 streamed)
Everything is fp16 on the wire with a positive rescaling chain (LAM_*)
keeping activations in fp16 range; PSUM accumulation is fp32.
"""

import numpy as np

import concourse.bass as bass
import concourse.mybir as mybir
import concourse.tile as tile
from concourse import bacc
from concourse.bass_utils import run_bass_kernel_spmd

F32 = mybir.dt.float32
F16 = mybir.dt.float16
AF = mybir.ActivationFunctionType

B, C, H, W = 256, 3, 64, 64
MODES = C * H * W              # 12288
NTOT = B * MODES
ALPHA = 0.5
NCORE = 8
BS = B // NCORE                # 32 batches per core
XCOLS = 33 * 96                # 3168 = 3072 pixel chunks + pad for the view

LAM_H, LAM_1, LAM_2 = 16.0, 4.0, 4.0


# ---------------------------------------------------------------- host folds
def _fold_weights(Ws1, bs1, Ws2, bs2, Wn1, bn1, Wn2, bn2, Wn3, bn3):
    f16 = lambda a: np.ascontiguousarray(a, dtype=np.float16)
    s = float(np.float64(1.0 / (NTOT - 1)) ** (-ALPHA))

    # GL weights w_j (enough taps for in-batch + 512-halo reach)
    j = np.arange(1, 13312, dtype=np.float64)
    wgl = np.concatenate([[1.0], np.cumprod((j - 1.0 - ALPHA) / j)])

    # V0[m] = sum_d w_d W1s[m+d];  V1h[m'] = sum_k w_{k+512-m'} W1s[k]
    L = 32768
    W1s = Ws1.astype(np.float64) * (s / LAM_H)
    corr = np.fft.irfft(
        np.fft.rfft(W1s, n=L, axis=0) * np.conj(np.fft.rfft(wgl, n=L))[:, None],
        n=L, axis=0)
    V0 = corr[:MODES].astype(np.float32)
    V1h = corr[L - 512:].astype(np.float32)

    jk = np.outer(np.arange(64), np.arange(64)).astype(np.float64)
    Cm = np.cos(2 * np.pi * jk / 64).astype(np.float32)
    Sm = np.sin(2 * np.pi * jk / 64).astype(np.float32)

    # V0_pix[(y,z),n] = sum_{u,v} (C[u,y]C[v,z] - S[u,y]S[v,z]) V0[(u,v),n]
    V0c = V0.reshape(3, 64, 64, 512)
    V0_pix = (np.einsum('uy,cuvn,vz->cyzn', Cm, V0c, Cm, optimize=True)
              - np.einsum('uy,cuvn,vz->cyzn', Sm, V0c, Sm, optimize=True)
              ).reshape(MODES, 512)
    Vcat = np.concatenate([V1h, V0_pix], axis=0)            # (12800, 512)

    U = (Ws2.astype(np.float32) @ Wn1.astype(np.float32)) * np.float32(LAM_H / LAM_1)
    cU = ((bs2.astype(np.float32) @ Wn1.astype(np.float32) + bn1)
          / np.float32(LAM_1))
    W4 = Wn2 * np.float32(LAM_1 / LAM_2)

    # W5f = (Wn3 o Re(iDFT)) * LAM_2 ; b5f = bn3 o Re(iDFT)
    W5c = Wn3.astype(np.float32).reshape(512, 3, 64, 64)
    W5f = ((np.einsum('rcuv,uy,vz->rcyz', W5c, Cm, Cm, optimize=True)
            - np.einsum('rcuv,uy,vz->rcyz', W5c, Sm, Sm, optimize=True))
           * np.float32(LAM_2 / 4096.0)).reshape(512, MODES)
    b5c = bn3.astype(np.float32).reshape(3, 64, 64)
    b5f = ((np.einsum('cuv,uy,vz->cyz', b5c, Cm, Cm, optimize=True)
            - np.einsum('cuv,uy,vz->cyz', b5c, Sm, Sm, optimize=True))
           / np.float32(4096.0)).reshape(MODES)

    return {
        "w1f": f16(Vcat.reshape(25, 4, 128, 512).transpose(0, 2, 1, 3)),
        "uT": f16(U.reshape(4, 128, 4, 128).transpose(2, 1, 0, 3)
                  .reshape(4, 128, 512).transpose(1, 0, 2)),
        "w4t": f16(W4.reshape(4, 128, 4, 128).transpose(2, 1, 0, 3)
                   .reshape(4, 128, 512).transpose(1, 0, 2)),
        "w5f": f16(W5f.reshape(4, 128, 12, 1024).transpose(2, 1, 0, 3)),
        "b1r": f16((bs1 / LAM_H).reshape(1, 512)),
        "cUt": np.ascontiguousarray(cU.reshape(4, 128).T, dtype=np.float32),
        "b4t": np.ascontiguousarray((bn2 / LAM_2).reshape(4, 128).T,
                                    dtype=np.float32),
        "b5r": f16(b5f.reshape(1, MODES)),
        "idn32": f16(np.eye(32)),
        "ones1": f16(np.ones((1, 32))),
    }


# ---------------------------------------------------------------- bass module
_NC_CACHE = None


def _build_nc():
    nc = bacc.Bacc("TRN2", target_bir_lowering=False, debug=False,
                   num_devices=NCORE)

    def din(name, shape, dt=F16):
        return nc.dram_tensor(name, shape, dt, kind="ExternalInput")

    d_xpix = din("xpix", (128, XCOLS))
    d_hgT = din("hgT", (128, 128))
    d_w1f = din("w1f", (25, 128, 4, 512))
    d_uT = din("uT", (128, 4, 512))
    d_w4t = din("w4t", (128, 4, 512))
    d_w5f = din("w5f", (12, 128, 4, 1024))
    d_b1 = din("b1r", (1, 512))
    d_cU = nc.dram_tensor("cUt", (128, 4), F32, kind="ExternalInput")
    d_b4 = nc.dram_tensor("b4t", (128, 4), F32, kind="ExternalInput")
    d_b5 = din("b5r", (1, MODES))
    d_idn = din("idn32", (32, 32))
    d_ones = din("ones1", (1, 32))
    d_out = nc.dram_tensor("out", (BS, MODES), F16, kind="ExternalOutput")

    with tile.TileContext(nc) as tc:
        with tc.tile_pool(name="cpool", bufs=1) as cpool:
            xpix = cpool.tile([128, XCOLS], F16, tag="xpix")
            hgT = cpool.tile([128, 128], F16, tag="hgT")
            idn = cpool.tile([32, 32], F16, tag="idn")
            ones1 = cpool.tile([1, 32], F16, tag="ones1")
            b1s = cpool.tile([1, 512], F16, tag="b1s")
            cUs = cpool.tile([128, 4], F32, tag="cUs")
            b4s = cpool.tile([128, 4], F32, tag="b4s")
            b5s = cpool.tile([1, MODES], F16, tag="b5s")
            uT = cpool.tile([128, 4, 512], F16, tag="uT")
            w4 = cpool.tile([128, 4, 512], F16, tag="w4")
            h_sb = cpool.tile([32, 512], F16, tag="h_sb")
            hT = cpool.tile([128, 4, 32], F16, tag="hT")
            h1T = cpool.tile([128, 4, 32], F16, tag="h1T")
            h2T = cpool.tile([128, 4, 32], F16, tag="h2T")

            nc.sync.dma_start(xpix[:], d_xpix[:])
            nc.sync.dma_start(hgT[:], d_hgT[:])
            for t, dref in ((idn, d_idn), (ones1, d_ones), (b1s, d_b1),
                            (cUs, d_cU), (b4s, d_b4)):
                nc.sync.dma_start(t[:], dref[:])

            vx = xpix.rearrange("p (b k) -> p b k", b=33)
            vh = hgT.rearrange("p (b k) -> p b k", b=32)

            # ======= L1: h = relu([halo|x] @ [V1h;V0_pix] + b1) =============
            with tc.tile_pool(name="wp", bufs=8) as wp, \
                 tc.tile_pool(name="ps1", bufs=1, space="PSUM") as ps1, \
                 tc.tile_pool(name="pst", bufs=1, space="PSUM") as pst:
                acc = ps1.tile([32, 512], F32, tag="acc")
                for K4 in range(25):
                    wt = wp.tile([128, 4, 512], F16, tag="wt")
                    nc.sync.dma_start(wt[:], d_w1f[K4])
                    for jj in range(4):
                        q = 4 * K4 + jj
                        src = vh[:, :, q] if q < 4 else vx[:, 0:32, q - 4]
                        nc.tensor.matmul(acc[:], src, wt[:, jj, :],
                                         start=(q == 0), stop=False)
                nc.tensor.matmul(acc[:], ones1[:], b1s[:],
                                 start=False, stop=True)
                nc.scalar.activation(h_sb[:], acc[:], AF.Relu)
                pt = pst.tile([128, 128], F16, tag="pt")
                for fb in range(4):
                    nc.tensor.transpose(pt[:, fb * 32:(fb + 1) * 32],
                                        h_sb[:, fb * 128:(fb + 1) * 128],
                                        idn[:])
                nc.vector.tensor_copy(hT[:],
                                      pt.rearrange("p (f b) -> p f b", f=4))

            # uT/w4t arrive after the w1f stream, before the w5f stream
            nc.sync.dma_start(uT[:], d_uT[:])
            nc.sync.dma_start(w4[:], d_w4t[:])
            nc.sync.dma_start(b5s[:], d_b5[:])

            # ======= L2 (U) and L4 (W4): weights-stationary 512->512 ========
            with tc.tile_pool(name="ps2", bufs=4, space="PSUM") as ps2:
                for wsb, bias, src, dst in ((uT, cUs, hT, h1T),
                                            (w4, b4s, h1T, h2T)):
                    for f2b in range(4):
                        acc2 = ps2.tile([128, 32], F32, tag="acc2")
                        for fb in range(4):
                            nc.tensor.matmul(
                                acc2[:], wsb[:, f2b, fb * 128:(fb + 1) * 128],
                                src[:, fb, :], start=(fb == 0), stop=(fb == 3))
                        nc.scalar.activation(dst[:, f2b, :], acc2[:], AF.Relu,
                                             bias=bias[:, f2b:f2b + 1])

            # ======= L5: out = h2 @ W5f + b5f ===============================
            with tc.tile_pool(name="wp5", bufs=4) as wp5, \
                 tc.tile_pool(name="sp5", bufs=4) as sp5, \
                 tc.tile_pool(name="ps5", bufs=4, space="PSUM") as ps5:
                for mc2 in range(12):
                    wt = wp5.tile([128, 4, 1024], F16, tag="w5")
                    nc.sync.dma_start(wt[:], d_w5f[mc2])
                    for half in range(2):
                        mc = 2 * mc2 + half
                        acc5 = ps5.tile([32, 512], F32, tag="acc5")
                        for fb in range(4):
                            nc.tensor.matmul(
                                acc5[:], h2T[:, fb, :],
                                wt[:, fb, half * 512:(half + 1) * 512],
                                start=(fb == 0), stop=False)
                        nc.tensor.matmul(acc5[:], ones1[:],
                                         b5s[0:1, mc * 512:(mc + 1) * 512],
                                         start=False, stop=True)
                        osb = sp5.tile([32, 512], F16, tag="osb")
                        if half == 0:
                            nc.scalar.copy(osb[:], acc5[:])
                        else:
                            nc.vector.tensor_copy(osb[:], acc5[:])
                        nc.gpsimd.dma_start(
                            d_out[:, mc * 512:(mc + 1) * 512], osb[:])

    nc.compile()
    return nc


def _get_nc():
    global _NC_CACHE
    if _NC_CACHE is None:
        _NC_CACHE = _build_nc()
    return _NC_CACHE


def _make_in_maps(x, Ws1, bs1, Ws2, bs2, Wn1, bn1, Wn2, bn2, Wn3, bn3):
    shared = _fold_weights(Ws1, bs1, Ws2, bs2, Wn1, bn1, Wn2, bn2, Wn3, bn3)

    # halo: last 512 DFT-real values of every channel-2 image
    hg_all = np.real(np.fft.fft2(x[:, 2]))[:, 56:64, :].reshape(B, 512)
    hg_all = hg_all.astype(np.float16)

    in_maps = []
    for g in range(NCORE):
        xc = x[g * BS:(g + 1) * BS].reshape(BS * MODES).astype(np.float16)
        xpix = np.zeros((128, XCOLS), np.float16)
        xpix[:, :BS * 96] = xc.reshape(BS * 96, 128).T
        hgT = np.zeros((128, 128), np.float16)
        for b in range(BS):
            gi = g * BS + b - 1
            if gi >= 0:
                hgT[:, 4 * b:4 * b + 4] = hg_all[gi].reshape(4, 128).T
        in_maps.append({"xpix": np.ascontiguousarray(xpix),
                        "hgT": np.ascontiguousarray(hgT), **shared})
    return in_maps


def kernel(**inputs):
    x = np.ascontiguousarray(inputs["x"], dtype=np.float32)
    nc = _get_nc()
    in_maps = _make_in_maps(
        x, inputs["Ws1"], inputs["bs1"], inputs["Ws2"], inputs["bs2"],
        inputs["Wn1"], inputs["bn1"], inputs["Wn2"], inputs["bn2"],
        inputs["Wn3"], inputs["bn3"])
    res = run_bass_kernel_spmd(nc, in_maps, list(range(NCORE)))
    out = np.empty((B, C, H, W), np.float32)
    for g in range(NCORE):
        out[g * BS:(g + 1) * BS] = (
            res.results[g]["out"].astype(np.float32).reshape(BS, C, H, W))
    return out


# revision 6
# speedup vs baseline: 2.8874x; 1.2705x over previous
"""Trainium2 Bass kernel for nn_EnhancedFractionalPINO.

Math folding (all precomputed on host, per call):
  reference out = iDFT( relu(relu(relu(GLconv(DFT(x))@Ws1+b1) @ (Ws2@Wn1)
                  + (bs2@Wn1+bn1)) @ Wn2 + bn2) @ Wn3 + bn3 )
  - Ws2@Wn1 folds to a single 512x512 matrix U (no relu between the two
    12288-wide matmuls in the reference), eliminating both of them.
  - The GL fractional conv (lower-triangular Toeplitz T0 within a batch row
    plus a 512-sample halo from the previous batch) and the forward 2-D DFT
    fold into Ws1:  V0_pix = D^T T0^T Ws1 acts on raw pixels;  a 512x512
    V1h acts on the last 512 DFT values of the previous batch's channel-2
    image (computed on host via fft2).  Full in-batch GL taps -> more
    accurate than any on-device truncation.
  - The inverse 2-D DFT folds into Wn3: W5f = Wn3 o blockdiag(Re(iDFT));
    the device's final matmul emits the finished result.

Per core (batch-parallel, 32 batches/core): a 4-layer MLP
  h   = relu([halo | x_pixels] @ [V1h; V0_pix] + b1)      (K=12800 streamed)
  h1  = relu(h @ U + cU);  h2 = relu(h1 @ W4 + b4)        (weights resident)
  out = h2 @ W5f + b5f                                    (N=12288 streamed)
Everything is fp16 on the wire with a positive rescaling chain (LAM_*)
keeping activations in fp16 range; PSUM accumulation is fp32.
"""

import numpy as np

import concourse.bass as bass
import concourse.mybir as mybir
import concourse.tile as tile
from concourse import bacc
from concourse.bass_utils import run_bass_kernel_spmd

F32 = mybir.dt.float32
F16 = mybir.dt.float16
F8 = mybir.dt.float8e3
AF = mybir.ActivationFunctionType

B, C, H, W = 256, 3, 64, 64
MODES = C * H * W              # 12288
NTOT = B * MODES
ALPHA = 0.5
NCORE = 8
BS = B // NCORE                # 32 batches per core
XCOLS = 33 * 96                # 3168 = 3072 pixel chunks + pad for the view

LAM_H, LAM_1, LAM_2 = 16.0, 4.0, 4.0


# ---------------------------------------------------------------- host folds
def _fold_weights(Ws1, bs1, Ws2, bs2, Wn1, bn1, Wn2, bn2, Wn3, bn3):
    f16 = lambda a: np.ascontiguousarray(a, dtype=np.float16)
    s = float(np.float64(1.0 / (NTOT - 1)) ** (-ALPHA))

    # GL weights w_j (enough taps for in-batch + 512-halo reach)
    j = np.arange(1, 13312, dtype=np.float64)
    wgl = np.concatenate([[1.0], np.cumprod((j - 1.0 - ALPHA) / j)])

    # V0[m] = sum_d w_d W1s[m+d];  V1h[m'] = sum_k w_{k+512-m'} W1s[k]
    L = 32768
    W1s = Ws1.astype(np.float64) * (s / LAM_H)
    corr = np.fft.irfft(
        np.fft.rfft(W1s, n=L, axis=0) * np.conj(np.fft.rfft(wgl, n=L))[:, None],
        n=L, axis=0)
    V0 = corr[:MODES].astype(np.float32)
    V1h = corr[L - 512:].astype(np.float32)

    jk = np.outer(np.arange(64), np.arange(64)).astype(np.float64)
    Cm = np.cos(2 * np.pi * jk / 64).astype(np.float32)
    Sm = np.sin(2 * np.pi * jk / 64).astype(np.float32)

    # V0_pix[(y,z),n] = sum_{u,v} (C[u,y]C[v,z] - S[u,y]S[v,z]) V0[(u,v),n]
    V0c = V0.reshape(3, 64, 64, 512)
    V0_pix = (np.einsum('uy,cuvn,vz->cyzn', Cm, V0c, Cm, optimize=True)
              - np.einsum('uy,cuvn,vz->cyzn', Sm, V0c, Sm, optimize=True)
              ).reshape(MODES, 512)
    Vcat = np.concatenate([V1h, V0_pix], axis=0)            # (12800, 512)

    # e3m4 per-chunk pow2 scaling: chunk q rows [128q,128q+128); the scale is
    # compensated exactly in the (disjoint) x / halo column groups.
    f83 = mybir.dt.np(F8)
    am = np.abs(Vcat.reshape(100, 128 * 512)).max(axis=1)
    kq = np.clip(np.floor(np.log2(15.5 / np.maximum(am, 1e-12))), -12, 12)
    wscale = (2.0 ** kq).astype(np.float32)
    Vq8 = (Vcat.reshape(100, 128, 512) * wscale[:, None, None]).astype(f83)
    xscale = (2.0 ** (-kq)).astype(np.float32)

    U = (Ws2.astype(np.float32) @ Wn1.astype(np.float32)) * np.float32(LAM_H / LAM_1)
    cU = ((bs2.astype(np.float32) @ Wn1.astype(np.float32) + bn1)
          / np.float32(LAM_1))
    W4 = Wn2 * np.float32(LAM_1 / LAM_2)

    # W5f = (Wn3 o Re(iDFT)) * LAM_2 ; b5f = bn3 o Re(iDFT)
    W5c = Wn3.astype(np.float32).reshape(512, 3, 64, 64)
    W5f = ((np.einsum('rcuv,uy,vz->rcyz', W5c, Cm, Cm, optimize=True)
            - np.einsum('rcuv,uy,vz->rcyz', W5c, Sm, Sm, optimize=True))
           * np.float32(LAM_2 / 4096.0)).reshape(512, MODES)
    b5c = bn3.astype(np.float32).reshape(3, 64, 64)
    b5f = ((np.einsum('cuv,uy,vz->cyz', b5c, Cm, Cm, optimize=True)
            - np.einsum('cuv,uy,vz->cyz', b5c, Sm, Sm, optimize=True))
           / np.float32(4096.0)).reshape(MODES)

    return {
        "w1f": np.ascontiguousarray(
            Vq8.reshape(25, 4, 128, 512).transpose(0, 2, 1, 3)),
        "_xscale": xscale,
        "uw": f16(np.concatenate(
            [U.reshape(4, 128, 4, 128).transpose(2, 1, 0, 3)
              .reshape(4, 128, 512).transpose(1, 0, 2),
             W4.reshape(4, 128, 4, 128).transpose(2, 1, 0, 3)
              .reshape(4, 128, 512).transpose(1, 0, 2)], axis=1)),
        "w5f": f16(W5f.reshape(4, 128, 12, 1024).transpose(2, 1, 0, 3)),
        "b1r": f16((bs1 / LAM_H).reshape(1, 512)),
        "cUt": np.ascontiguousarray(cU.reshape(4, 128).T, dtype=np.float32),
        "b4t": np.ascontiguousarray((bn2 / LAM_2).reshape(4, 128).T,
                                    dtype=np.float32),
        "b5r": f16(b5f.reshape(1, MODES)),
        "idn32": f16(np.eye(32)),
        "ones1": f16(np.ones((1, 32))),
    }


# ---------------------------------------------------------------- bass module
_NC_CACHE = None


def _build_nc():
    nc = bacc.Bacc("TRN2", target_bir_lowering=False, debug=False,
                   num_devices=NCORE)

    def din(name, shape, dt=F16):
        return nc.dram_tensor(name, shape, dt, kind="ExternalInput")

    d_xpix = din("xpix", (128, XCOLS))
    d_hgT = din("hgT", (128, 128))
    d_w1f = din("w1f", (25, 128, 4, 512), F8)
    d_uw = din("uw", (128, 8, 512))
    d_w5f = din("w5f", (12, 128, 4, 1024))
    d_b1 = din("b1r", (1, 512))
    d_cU = nc.dram_tensor("cUt", (128, 4), F32, kind="ExternalInput")
    d_b4 = nc.dram_tensor("b4t", (128, 4), F32, kind="ExternalInput")
    d_b5 = din("b5r", (1, MODES))
    d_idn = din("idn32", (32, 32))
    d_ones = din("ones1", (1, 32))
    d_out = nc.dram_tensor("out", (BS, MODES), F16, kind="ExternalOutput")

    with tile.TileContext(nc) as tc:
        with tc.tile_pool(name="cpool", bufs=1) as cpool:
            xpix = cpool.tile([128, XCOLS], F16, tag="xpix")
            hgT = cpool.tile([128, 128], F16, tag="hgT")
            idn = cpool.tile([32, 32], F16, tag="idn")
            ones1 = cpool.tile([1, 32], F16, tag="ones1")
            b1s = cpool.tile([1, 512], F16, tag="b1s")
            cUs = cpool.tile([128, 4], F32, tag="cUs")
            b4s = cpool.tile([128, 4], F32, tag="b4s")
            b5s = cpool.tile([1, MODES], F16, tag="b5s")
            uw = cpool.tile([128, 8, 512], F16, tag="uw")
            h_sb = cpool.tile([32, 512], F16, tag="h_sb")
            hT = cpool.tile([128, 4, 32], F16, tag="hT")
            h1T = cpool.tile([128, 4, 32], F16, tag="h1T")
            h2T = cpool.tile([128, 4, 32], F16, tag="h2T")

            nc.sync.dma_start(xpix[:], d_xpix[:])
            nc.sync.dma_start(hgT[:], d_hgT[:])
            for t, dref in ((idn, d_idn), (ones1, d_ones), (b1s, d_b1),
                            (cUs, d_cU), (b4s, d_b4)):
                nc.scalar.dma_start(t[:], dref[:])

            vx = xpix.rearrange("p (b k) -> p b k", b=33)
            vh = hgT.rearrange("p (b k) -> p b k", b=32)

            # ======= L1: h = relu([halo|x] @ [V1h;V0_pix] + b1) =============
            with tc.tile_pool(name="wp", bufs=8) as wp, \
                 tc.tile_pool(name="ps1", bufs=1, space="PSUM") as ps1, \
                 tc.tile_pool(name="pst", bufs=1, space="PSUM") as pst:
                acc = ps1.tile([32, 512], F32, tag="acc")
                for K4 in range(25):
                    wt = wp.tile([128, 4, 512], F8, tag="wt")
                    nc.sync.dma_start(wt[:], d_w1f[K4])
                    for jj in range(4):
                        q = 4 * K4 + jj
                        src = vh[:, :, q] if q < 4 else vx[:, 0:32, q - 4]
                        nc.tensor.matmul(acc[:], src, wt[:, jj, :],
                                         start=(q == 0), stop=False)
                nc.tensor.matmul(acc[:], ones1[:], b1s[:],
                                 start=False, stop=True)
                nc.scalar.activation(h_sb[:], acc[:], AF.Relu)
                pt = pst.tile([128, 128], F16, tag="pt")
                for fb in range(4):
                    nc.tensor.transpose(pt[:, fb * 32:(fb + 1) * 32],
                                        h_sb[:, fb * 128:(fb + 1) * 128],
                                        idn[:])
                nc.vector.tensor_copy(hT[:],
                                      pt.rearrange("p (f b) -> p f b", f=4))

            nc.sync.dma_start(uw[:], d_uw[:])
            nc.sync.dma_start(b5s[:], d_b5[:])

            # ======= L2 (U) and L4 (W4): weights-stationary 512->512 ========
            with tc.tile_pool(name="ps2", bufs=4, space="PSUM") as ps2:
                for wo, bias, src, dst in ((0, cUs, hT, h1T),
                                           (4, b4s, h1T, h2T)):
                    for f2b in range(4):
                        acc2 = ps2.tile([128, 32], F32, tag="acc2")
                        for fb in range(4):
                            nc.tensor.matmul(
                                acc2[:],
                                uw[:, wo + f2b, fb * 128:(fb + 1) * 128],
                                src[:, fb, :], start=(fb == 0), stop=(fb == 3))
                        nc.scalar.activation(dst[:, f2b, :], acc2[:], AF.Relu,
                                             bias=bias[:, f2b:f2b + 1])

            # ======= L5: out = h2 @ W5f + b5f ===============================
            with tc.tile_pool(name="wp5", bufs=4) as wp5, \
                 tc.tile_pool(name="sp5", bufs=8) as sp5, \
                 tc.tile_pool(name="ps5", bufs=8, space="PSUM") as ps5:
                for mc2 in range(12):
                    wt = wp5.tile([128, 4, 1024], F16, tag="w5")
                    if mc2 < 11:
                        nc.sync.dma_start(wt[:], d_w5f[mc2])
                    else:
                        nc.sync.dma_start(wt[:, :, 0:512],
                                          d_w5f[mc2][:, :, 0:512])
                        nc.sync.dma_start(wt[:, :, 512:1024],
                                          d_w5f[mc2][:, :, 512:1024])
                    for half in range(2):
                        mc = 2 * mc2 + half
                        acc5 = ps5.tile([32, 512], F32, tag="acc5")
                        for fb in range(4):
                            nc.tensor.matmul(
                                acc5[:], h2T[:, fb, :],
                                wt[:, fb, half * 512:(half + 1) * 512],
                                start=(fb == 0), stop=False)
                        nc.tensor.matmul(acc5[:], ones1[:],
                                         b5s[0:1, mc * 512:(mc + 1) * 512],
                                         start=False, stop=True)
                        if half == 0:
                            osb = sp5.tile([32, 1024], F16, tag="osb")
                            nc.scalar.copy(osb[:, 0:512], acc5[:])
                        else:
                            nc.vector.tensor_copy(osb[:, 512:1024], acc5[:])
                            nc.gpsimd.dma_start(
                                d_out[:, mc2 * 1024:(mc2 + 1) * 1024], osb[:])

    nc.compile()
    return nc


def _get_nc():
    global _NC_CACHE
    if _NC_CACHE is None:
        _NC_CACHE = _build_nc()
    return _NC_CACHE


def _make_in_maps(x, Ws1, bs1, Ws2, bs2, Wn1, bn1, Wn2, bn2, Wn3, bn3):
    shared = _fold_weights(Ws1, bs1, Ws2, bs2, Wn1, bn1, Wn2, bn2, Wn3, bn3)
    xscale = shared.pop("_xscale")

    # halo: last 512 DFT-real values of every channel-2 image
    hg_all = np.real(np.fft.fft2(x[:, 2]))[:, 56:64, :].reshape(B, 512)
    hg_all = (hg_all.reshape(B, 4, 128)
              * xscale[0:4][None, :, None]).astype(np.float16)

    in_maps = []
    for g in range(NCORE):
        xc = (x[g * BS:(g + 1) * BS].reshape(BS, 96, 128)
              * xscale[None, 4:, None]).astype(np.float16)
        xpix = np.zeros((128, XCOLS), np.float16)
        xpix[:, :BS * 96] = xc.reshape(BS * 96, 128).T
        hgT = np.zeros((128, 128), np.float16)
        for b in range(BS):
            gi = g * BS + b - 1
            if gi >= 0:
                hgT[:, 4 * b:4 * b + 4] = hg_all[gi].T
        in_maps.append({"xpix": np.ascontiguousarray(xpix),
                        "hgT": np.ascontiguousarray(hgT), **shared})
    return in_maps


def kernel(**inputs):
    x = np.ascontiguousarray(inputs["x"], dtype=np.float32)
    nc = _get_nc()
    in_maps = _make_in_maps(
        x, inputs["Ws1"], inputs["bs1"], inputs["Ws2"], inputs["bs2"],
        inputs["Wn1"], inputs["bn1"], inputs["Wn2"], inputs["bn2"],
        inputs["Wn3"], inputs["bn3"])
    res = run_bass_kernel_spmd(nc, in_maps, list(range(NCORE)))
    out = np.empty((B, C, H, W), np.float32)
    for g in range(NCORE):
        out[g * BS:(g + 1) * BS] = (
            res.results[g]["out"].astype(np.float32).reshape(BS, C, H, W))
    return out


# revision 7
# speedup vs baseline: 3.3521x; 1.1609x over previous
"""Trainium2 Bass kernel for nn_EnhancedFractionalPINO.

Math folding (all precomputed on host, per call):
  reference out = iDFT( relu(relu(relu(GLconv(DFT(x))@Ws1+b1) @ (Ws2@Wn1)
                  + (bs2@Wn1+bn1)) @ Wn2 + bn2) @ Wn3 + bn3 )
  - Ws2@Wn1 folds to a single 512x512 matrix U (no relu between the two
    12288-wide matmuls in the reference), eliminating both of them.
  - The GL fractional conv (lower-triangular Toeplitz T0 within a batch row
    plus a 512-sample halo from the previous batch) and the forward 2-D DFT
    fold into Ws1:  V0_pix = D^T T0^T Ws1 acts on raw pixels;  a 512x512
    V1h acts on the last 512 DFT values of the previous batch's channel-2
    image (computed on host via fft2).  Full in-batch GL taps.
  - The inverse 2-D DFT folds into Wn3: W5f = Wn3 o blockdiag(Re(iDFT)).
  - The final bias (b5f = bn3 o iDFT) is added on the host (linear).

Precision: L1 weights are float8-e3m4 with a per-chunk pow2 scale compensated
exactly in the disjoint x / halo column groups; the last K-quarter of W5f is
e3m4 with a pow2 scale compensated in h2's fb=3 block (relu commutes with
positive scales).  All other tensors fp16; PSUM accumulation fp32.

Per core (batch-parallel, 32 batches/core): a 4-layer MLP
  h   = relu([halo | x_pixels] @ [V1h; V0_pix] + b1)      (K=12800 streamed)
  h1  = relu(h @ U + cU);  h2 = relu(h1 @ W4 + b4)        (weights resident)
  out = h2 @ W5f                                          (N=12288 streamed)
"""

import numpy as np

import concourse.bass as bass
import concourse.mybir as mybir
import concourse.tile as tile
from concourse import bacc
from concourse.bass_utils import run_bass_kernel_spmd

F32 = mybir.dt.float32
F16 = mybir.dt.float16
F8 = mybir.dt.float8e3
AF = mybir.ActivationFunctionType

B, C, H, W = 256, 3, 64, 64
MODES = C * H * W              # 12288
NTOT = B * MODES
ALPHA = 0.5
NCORE = 8
BS = B // NCORE                # 32 batches per core
XCOLS = 33 * 96                # 3168 = 3072 pixel chunks + pad for the view

LAM_H, LAM_1, LAM_2 = 16.0, 4.0, 4.0
K5 = 11                        # pow2 scale exponent for the fp8 block of W5f


# ---------------------------------------------------------------- host folds
def _fold_weights(Ws1, bs1, Ws2, bs2, Wn1, bn1, Wn2, bn2, Wn3, bn3):
    f16 = lambda a: np.ascontiguousarray(a, dtype=np.float16)
    f83 = mybir.dt.np(F8)
    s = float(np.float64(1.0 / (NTOT - 1)) ** (-ALPHA))

    # GL weights w_j (enough taps for in-batch + 512-halo reach)
    j = np.arange(1, 13312, dtype=np.float64)
    wgl = np.concatenate([[1.0], np.cumprod((j - 1.0 - ALPHA) / j)])

    # V0[m] = sum_d w_d W1s[m+d];  V1h[m'] = sum_k w_{k+512-m'} W1s[k]
    L = 32768
    W1s = Ws1.astype(np.float64) * (s / LAM_H)
    corr = np.fft.irfft(
        np.fft.rfft(W1s, n=L, axis=0) * np.conj(np.fft.rfft(wgl, n=L))[:, None],
        n=L, axis=0)
    V0 = corr[:MODES].astype(np.float32)
    V1h = corr[L - 512:].astype(np.float32)

    jk = np.outer(np.arange(64), np.arange(64)).astype(np.float64)
    Cm = np.cos(2 * np.pi * jk / 64).astype(np.float32)
    Sm = np.sin(2 * np.pi * jk / 64).astype(np.float32)

    # V0_pix[(y,z),n] = sum_{u,v} (C[u,y]C[v,z] - S[u,y]S[v,z]) V0[(u,v),n]
    V0c = V0.reshape(3, 64, 64, 512)
    V0_pix = (np.einsum('uy,cuvn,vz->cyzn', Cm, V0c, Cm, optimize=True)
              - np.einsum('uy,cuvn,vz->cyzn', Sm, V0c, Sm, optimize=True)
              ).reshape(MODES, 512)
    Vcat = np.concatenate([V1h, V0_pix], axis=0)            # (12800, 512)

    # e3m4 per-chunk pow2 scaling; the scale is compensated exactly in the
    # (disjoint) x / halo column groups.
    am = np.abs(Vcat.reshape(100, 128 * 512)).max(axis=1)
    kq = np.clip(np.floor(np.log2(15.5 / np.maximum(am, 1e-12))), -12, 12)
    Vq8 = (Vcat.reshape(100, 128, 512)
           * (2.0 ** kq)[:, None, None].astype(np.float32)).astype(f83)
    xscale = (2.0 ** (-kq)).astype(np.float32)

    U = (Ws2.astype(np.float32) @ Wn1.astype(np.float32)) * np.float32(LAM_H / LAM_1)
    cU = ((bs2.astype(np.float32) @ Wn1.astype(np.float32) + bn1)
          / np.float32(LAM_1))
    W4 = Wn2 * np.float32(LAM_1 / LAM_2)

    # W5f = (Wn3 o Re(iDFT)) * LAM_2 ; b5f = bn3 o Re(iDFT)  (host-added)
    W5c = Wn3.astype(np.float32).reshape(512, 3, 64, 64)
    W5f = ((np.einsum('rcuv,uy,vz->rcyz', W5c, Cm, Cm, optimize=True)
            - np.einsum('rcuv,uy,vz->rcyz', W5c, Sm, Sm, optimize=True))
           * np.float32(LAM_2 / 4096.0)).reshape(512, MODES)
    b5c = bn3.astype(np.float32).reshape(3, 64, 64)
    b5f = ((np.einsum('cuv,uy,vz->cyz', b5c, Cm, Cm, optimize=True)
            - np.einsum('cuv,uy,vz->cyz', b5c, Sm, Sm, optimize=True))
           / np.float32(4096.0)).reshape(MODES)

    # split W5f: K-blocks fb=0..2 fp16, fb=3 e3m4 scaled by 2^K5 (compensated
    # via h2's fb=3 block, scaled 2^-K5 at the L4 activation)
    W5r = W5f.reshape(4, 128, 12, 1024)
    assert np.abs(W5r[3]).max() * 2.0 ** K5 <= 15.5
    b4t = (bn2 / LAM_2).reshape(4, 128).T.astype(np.float32).copy()
    b4t[:, 3] *= np.float32(2.0 ** (-K5))

    return {
        "w1f": np.ascontiguousarray(
            Vq8.reshape(25, 4, 128, 512).transpose(0, 2, 1, 3)),
        "_xscale": xscale,
        "_b5f": b5f,
        "uw": f16(np.concatenate(
            [U.reshape(4, 128, 4, 128).transpose(2, 1, 0, 3)
              .reshape(4, 128, 512).transpose(1, 0, 2),
             W4.reshape(4, 128, 4, 128).transpose(2, 1, 0, 3)
              .reshape(4, 128, 512).transpose(1, 0, 2)], axis=1)),
        "w5f16": f16(W5r[0:3].transpose(2, 1, 0, 3)),
        "w5f8": np.ascontiguousarray(
            (W5r[3] * np.float32(2.0 ** K5)).transpose(1, 0, 2)
            .astype(f83)),
        "b1t": np.ascontiguousarray((bs1 / LAM_H).reshape(4, 128).T,
                                    dtype=np.float32),
        "cUt": np.ascontiguousarray(cU.reshape(4, 128).T, dtype=np.float32),
        "b4t": np.ascontiguousarray(b4t),
        "idn32": f16(np.eye(32)),
        "ones1": f16(np.ones((1, 32))),
    }


# ---------------------------------------------------------------- bass module
_NC_CACHE = None


def _build_nc():
    nc = bacc.Bacc("TRN2", target_bir_lowering=False, debug=False,
                   num_devices=NCORE)

    def din(name, shape, dt=F16):
        return nc.dram_tensor(name, shape, dt, kind="ExternalInput")

    d_xpix = din("xpix", (128, XCOLS))
    d_hgT = din("hgT", (128, 128))
    d_w1f = din("w1f", (25, 128, 4, 512), F8)
    d_uw = din("uw", (128, 8, 512))
    d_w5f16 = din("w5f16", (12, 128, 3, 1024))
    d_w5f8 = din("w5f8", (12, 128, 1024), F8)
    d_b1 = nc.dram_tensor("b1t", (128, 4), F32, kind="ExternalInput")
    d_cU = nc.dram_tensor("cUt", (128, 4), F32, kind="ExternalInput")
    d_b4 = nc.dram_tensor("b4t", (128, 4), F32, kind="ExternalInput")
    d_out = nc.dram_tensor("out", (BS, MODES), F16, kind="ExternalOutput")

    with tile.TileContext(nc) as tc:
        with tc.tile_pool(name="cpool", bufs=1) as cpool, \
             tc.tile_pool(name="wp", bufs=8) as wp, \
             tc.tile_pool(name="wp5", bufs=6) as wp5, \
             tc.tile_pool(name="sp5", bufs=8) as sp5:
            xpix = cpool.tile([128, XCOLS], F16, tag="xpix")
            hgT = cpool.tile([128, 128], F16, tag="hgT")
            b1t = cpool.tile([128, 4], F32, tag="b1t")
            cUs = cpool.tile([128, 4], F32, tag="cUs")
            b4s = cpool.tile([128, 4], F32, tag="b4s")
            uw = cpool.tile([128, 8, 512], F16, tag="uw")
            hT = cpool.tile([128, 4, 32], F16, tag="hT")
            h1T = cpool.tile([128, 4, 32], F16, tag="h1T")
            h2T = cpool.tile([128, 4, 32], F16, tag="h2T")

            # big stream on sync; small consts on gpsimd (SWDGE: no HWDGE
            # contention with the stream's descriptor generation)
            nc.sync.dma_start(xpix[:], d_xpix[:])
            nc.sync.dma_start(hgT[:], d_hgT[:])
            for t, dref in ((b1t, d_b1), (cUs, d_cU), (b4s, d_b4)):
                nc.gpsimd.dma_start(t[:], dref[:])

            vx = xpix.rearrange("p (b k) -> p b k", b=33)
            vh = hgT.rearrange("p (b k) -> p b k", b=32)

            # ======= L1: hT = relu(W^T [halo|x]^T + b1), computed directly
            # in transposed form: weights are the stationary operand (out ap
            # is only 32), so no transposes and a per-partition bias.
            with tc.tile_pool(name="ps1", bufs=1, space="PSUM") as ps1:
                accs = [ps1.tile([128, 32], F32, tag=f"acc{nb}",
                                 name=f"acc{nb}") for nb in range(4)]
                for K4 in range(25):
                    wt = wp.tile([128, 4, 512], F8, tag="wt")
                    nc.sync.dma_start(wt[:], d_w1f[K4])
                    for jj in range(4):
                        q = 4 * K4 + jj
                        src = vh[:, :, q] if q < 4 else vx[:, 0:32, q - 4]
                        for nb in range(4):
                            nc.tensor.matmul(
                                accs[nb][:],
                                wt[:, jj, nb * 128:(nb + 1) * 128], src,
                                start=(q == 0), stop=(q == 99))
                # uw rides the stream right after w1f (inside the pool scope
                # so no released-zone barrier blocks it)
                nc.sync.dma_start(uw[:], d_uw[:])
                for nb in range(4):
                    nc.scalar.activation(hT[:, nb, :], accs[nb][:], AF.Relu,
                                         bias=b1t[:, nb:nb + 1])

            # ======= L2 (U) and L4 (W4): weights-stationary 512->512 ========
            # h2's fb=3 block carries the 2^-K5 compensation for the fp8
            # quarter of W5f (relu commutes; b4t col 3 is pre-scaled).
            with tc.tile_pool(name="ps2", bufs=4, space="PSUM") as ps2:
                for wo, bias, src, dst in ((0, cUs, hT, h1T),
                                           (4, b4s, h1T, h2T)):
                    for f2b in range(4):
                        acc2 = ps2.tile([128, 32], F32, tag="acc2")
                        for fb in range(4):
                            nc.tensor.matmul(
                                acc2[:],
                                uw[:, wo + f2b, fb * 128:(fb + 1) * 128],
                                src[:, fb, :], start=(fb == 0), stop=(fb == 3))
                        scl = (2.0 ** (-K5)) if (wo == 4 and f2b == 3) else 1.0
                        nc.scalar.activation(dst[:, f2b, :], acc2[:], AF.Relu,
                                             bias=bias[:, f2b:f2b + 1],
                                             scale=scl)

            # ======= L5: out = h2 @ W5f  (bias added on host) ===============
            with tc.tile_pool(name="ps5", bufs=8, space="PSUM") as ps5:
                for mc2 in range(12):
                    wt16 = wp5.tile([128, 3, 1024], F16, tag="w5a")
                    wt8 = wp5.tile([128, 1024], F8, tag="w5b")
                    if mc2 < 11:
                        nc.sync.dma_start(wt16[:], d_w5f16[mc2])
                        nc.sync.dma_start(wt8[:], d_w5f8[mc2])
                    else:
                        nc.sync.dma_start(wt16[:, :, 0:512],
                                          d_w5f16[mc2][:, :, 0:512])
                        nc.sync.dma_start(wt8[:], d_w5f8[mc2])
                        nc.sync.dma_start(wt16[:, :, 512:1024],
                                          d_w5f16[mc2][:, :, 512:1024])
                    for half in range(2):
                        mc = 2 * mc2 + half
                        acc5 = ps5.tile([32, 512], F32, tag="acc5")
                        for fb in range(3):
                            nc.tensor.matmul(
                                acc5[:], h2T[:, fb, :],
                                wt16[:, fb, half * 512:(half + 1) * 512],
                                start=(fb == 0), stop=False)
                        nc.tensor.matmul(
                            acc5[:], h2T[:, 3, :],
                            wt8[:, half * 512:(half + 1) * 512],
                            start=False, stop=True)
                        if half == 0:
                            osb = sp5.tile([32, 1024], F16, tag="osb")
                            nc.scalar.copy(osb[:, 0:512], acc5[:])
                        else:
                            nc.vector.tensor_copy(osb[:, 512:1024], acc5[:])
                            nc.gpsimd.dma_start(
                                d_out[:, mc2 * 1024:(mc2 + 1) * 1024], osb[:])

    nc.compile()
    return nc


def _get_nc():
    global _NC_CACHE
    if _NC_CACHE is None:
        _NC_CACHE = _build_nc()
    return _NC_CACHE


def _make_in_maps(x, Ws1, bs1, Ws2, bs2, Wn1, bn1, Wn2, bn2, Wn3, bn3):
    shared = _fold_weights(Ws1, bs1, Ws2, bs2, Wn1, bn1, Wn2, bn2, Wn3, bn3)
    xscale = shared.pop("_xscale")
    b5f = shared.pop("_b5f")

    # halo: last 512 DFT-real values of every channel-2 image
    hg_all = np.real(np.fft.fft2(x[:, 2]))[:, 56:64, :].reshape(B, 512)
    hg_all = (hg_all.reshape(B, 4, 128)
              * xscale[0:4][None, :, None]).astype(np.float16)

    in_maps = []
    for g in range(NCORE):
        xc = (x[g * BS:(g + 1) * BS].reshape(BS, 96, 128)
              * xscale[None, 4:, None]).astype(np.float16)
        xpix = np.zeros((128, XCOLS), np.float16)
        xpix[:, :BS * 96] = xc.reshape(BS * 96, 128).T
        hgT = np.zeros((128, 128), np.float16)
        for b in range(BS):
            gi = g * BS + b - 1
            if gi >= 0:
                hgT[:, 4 * b:4 * b + 4] = hg_all[gi].T
        in_maps.append({"xpix": np.ascontiguousarray(xpix),
                        "hgT": np.ascontiguousarray(hgT), **shared})
    return in_maps, b5f


def kernel(**inputs):
    x = np.ascontiguousarray(inputs["x"], dtype=np.float32)
    nc = _get_nc()
    in_maps, b5f = _make_in_maps(
        x, inputs["Ws1"], inputs["bs1"], inputs["Ws2"], inputs["bs2"],
        inputs["Wn1"], inputs["bn1"], inputs["Wn2"], inputs["bn2"],
        inputs["Wn3"], inputs["bn3"])
    res = run_bass_kernel_spmd(nc, in_maps, list(range(NCORE)))
    out = np.empty((B, C, H, W), np.float32)
    for g in range(NCORE):
        out[g * BS:(g + 1) * BS] = (
            (res.results[g]["out"].astype(np.float32) + b5f)
            .reshape(BS, C, H, W))
    return out


# revision 8
# speedup vs baseline: 3.4206x; 1.0204x over previous
"""Trainium2 Bass kernel for nn_EnhancedFractionalPINO.

Math folding (all precomputed on host, per call):
  reference out = iDFT( relu(relu(relu(GLconv(DFT(x))@Ws1+b1) @ (Ws2@Wn1)
                  + (bs2@Wn1+bn1)) @ Wn2 + bn2) @ Wn3 + bn3 )
  - Ws2@Wn1 folds to a single 512x512 matrix U (no relu between the two
    12288-wide matmuls in the reference), eliminating both of them.
  - The GL fractional conv (lower-triangular Toeplitz T0 within a batch row
    plus a 512-sample halo from the previous batch) and the forward 2-D DFT
    fold into Ws1:  V0_pix = D^T T0^T Ws1 acts on raw pixels;  a 512x512
    V1h acts on the last 512 DFT values of the previous batch's channel-2
    image (computed on host via fft2).  Full in-batch GL taps.
  - The inverse 2-D DFT folds into Wn3: W5f = Wn3 o blockdiag(Re(iDFT)).
  - The final bias (b5f = bn3 o iDFT) is added on the host (linear).

Precision: L1 weights are float8-e3m4 with a per-chunk pow2 scale compensated
exactly in the disjoint x / halo column groups; the last K-quarter of W5f is
e3m4 with a pow2 scale compensated in h2's fb=3 block (relu commutes with
positive scales).  All other tensors fp16; PSUM accumulation fp32.

Per core (batch-parallel, 32 batches/core): a 4-layer MLP
  h   = relu([halo | x_pixels] @ [V1h; V0_pix] + b1)      (K=12800 streamed)
  h1  = relu(h @ U + cU);  h2 = relu(h1 @ W4 + b4)        (weights resident)
  out = h2 @ W5f                                          (N=12288 streamed)
"""

import numpy as np

import concourse.bass as bass
import concourse.mybir as mybir
import concourse.tile as tile
from concourse import bacc
from concourse.bass_utils import run_bass_kernel_spmd

F32 = mybir.dt.float32
F16 = mybir.dt.float16
F8 = mybir.dt.float8e3
AF = mybir.ActivationFunctionType

B, C, H, W = 256, 3, 64, 64
MODES = C * H * W              # 12288
NTOT = B * MODES
ALPHA = 0.5
NCORE = 8
BS = B // NCORE                # 32 batches per core
XCOLS = 33 * 96                # 3168 = 3072 pixel chunks + pad for the view

LAM_H, LAM_1, LAM_2 = 16.0, 4.0, 4.0
K5 = 11                        # pow2 scale exponent for the fp8 block of W5f


# ---------------------------------------------------------------- host folds
def _fold_weights(Ws1, bs1, Ws2, bs2, Wn1, bn1, Wn2, bn2, Wn3, bn3):
    f16 = lambda a: np.ascontiguousarray(a, dtype=np.float16)
    f83 = mybir.dt.np(F8)
    s = float(np.float64(1.0 / (NTOT - 1)) ** (-ALPHA))

    # GL weights w_j (enough taps for in-batch + 512-halo reach)
    j = np.arange(1, 13312, dtype=np.float64)
    wgl = np.concatenate([[1.0], np.cumprod((j - 1.0 - ALPHA) / j)])

    # V0[m] = sum_d w_d W1s[m+d];  V1h[m'] = sum_k w_{k+512-m'} W1s[k]
    L = 32768
    W1s = Ws1.astype(np.float64) * (s / LAM_H)
    corr = np.fft.irfft(
        np.fft.rfft(W1s, n=L, axis=0) * np.conj(np.fft.rfft(wgl, n=L))[:, None],
        n=L, axis=0)
    V0 = corr[:MODES].astype(np.float32)
    V1h = corr[L - 512:].astype(np.float32)

    jk = np.outer(np.arange(64), np.arange(64)).astype(np.float64)
    Cm = np.cos(2 * np.pi * jk / 64).astype(np.float32)
    Sm = np.sin(2 * np.pi * jk / 64).astype(np.float32)

    # V0_pix[(y,z),n] = sum_{u,v} (C[u,y]C[v,z] - S[u,y]S[v,z]) V0[(u,v),n]
    V0c = V0.reshape(3, 64, 64, 512)
    V0_pix = (np.einsum('uy,cuvn,vz->cyzn', Cm, V0c, Cm, optimize=True)
              - np.einsum('uy,cuvn,vz->cyzn', Sm, V0c, Sm, optimize=True)
              ).reshape(MODES, 512)
    Vcat = np.concatenate([V1h, V0_pix], axis=0)            # (12800, 512)

    # e3m4 per-chunk pow2 scaling; the scale is compensated exactly in the
    # (disjoint) x / halo column groups.
    am = np.abs(Vcat.reshape(100, 128 * 512)).max(axis=1)
    kq = np.clip(np.floor(np.log2(15.5 / np.maximum(am, 1e-12))), -12, 12)
    Vq8 = (Vcat.reshape(100, 128, 512)
           * (2.0 ** kq)[:, None, None].astype(np.float32)).astype(f83)
    xscale = (2.0 ** (-kq)).astype(np.float32)

    U = (Ws2.astype(np.float32) @ Wn1.astype(np.float32)) * np.float32(LAM_H / LAM_1)
    cU = ((bs2.astype(np.float32) @ Wn1.astype(np.float32) + bn1)
          / np.float32(LAM_1))
    W4 = Wn2 * np.float32(LAM_1 / LAM_2)

    # W5f = (Wn3 o Re(iDFT)) * LAM_2 ; b5f = bn3 o Re(iDFT)  (host-added)
    W5c = Wn3.astype(np.float32).reshape(512, 3, 64, 64)
    W5f = ((np.einsum('rcuv,uy,vz->rcyz', W5c, Cm, Cm, optimize=True)
            - np.einsum('rcuv,uy,vz->rcyz', W5c, Sm, Sm, optimize=True))
           * np.float32(LAM_2 / 4096.0)).reshape(512, MODES)
    b5c = bn3.astype(np.float32).reshape(3, 64, 64)
    b5f = ((np.einsum('cuv,uy,vz->cyz', b5c, Cm, Cm, optimize=True)
            - np.einsum('cuv,uy,vz->cyz', b5c, Sm, Sm, optimize=True))
           / np.float32(4096.0)).reshape(MODES)

    # split W5f: K-blocks fb=0..2 fp16, fb=3 e3m4 scaled by 2^K5 (compensated
    # via h2's fb=3 block, scaled 2^-K5 at the L4 activation)
    W5r = W5f.reshape(4, 128, 12, 1024)
    lim = np.float32(15.5 * 2.0 ** (-K5))
    W5r = np.concatenate([W5r[0:3], np.clip(W5r[3:4], -lim, lim)])
    b4t = (bn2 / LAM_2).reshape(4, 128).T.astype(np.float32).copy()
    b4t[:, 3] *= np.float32(2.0 ** (-K5))

    return {
        "w1f": np.ascontiguousarray(
            Vq8.reshape(25, 4, 128, 512).transpose(0, 2, 1, 3)),
        "_xscale": xscale,
        "_b5f": b5f,
        "uw": f16(np.concatenate(
            [U.reshape(4, 128, 4, 128).transpose(2, 1, 0, 3)
              .reshape(4, 128, 512).transpose(1, 0, 2),
             W4.reshape(4, 128, 4, 128).transpose(2, 1, 0, 3)
              .reshape(4, 128, 512).transpose(1, 0, 2)], axis=1)),
        "w5f16": f16(W5r[0:3].transpose(2, 1, 0, 3)),
        "w5f8": np.ascontiguousarray(
            (W5r[3] * np.float32(2.0 ** K5)).transpose(1, 0, 2)
            .astype(f83)),
        "b1t": np.ascontiguousarray((bs1 / LAM_H).reshape(4, 128).T,
                                    dtype=np.float32),
        "cUt": np.ascontiguousarray(cU.reshape(4, 128).T, dtype=np.float32),
        "b4t": np.ascontiguousarray(b4t),
        "idn32": f16(np.eye(32)),
        "ones1": f16(np.ones((1, 32))),
    }


# ---------------------------------------------------------------- bass module
_NC_CACHE = None


def _build_nc():
    nc = bacc.Bacc("TRN2", target_bir_lowering=False, debug=False,
                   num_devices=NCORE)

    def din(name, shape, dt=F16):
        return nc.dram_tensor(name, shape, dt, kind="ExternalInput")

    d_xpix = din("xpix", (128, XCOLS))
    d_hgT = din("hgT", (128, 128))
    d_w1f = din("w1f", (25, 128, 4, 512), F8)
    d_uw = din("uw", (128, 8, 512))
    d_w5f16 = din("w5f16", (12, 128, 3, 1024))
    d_w5f8 = din("w5f8", (12, 128, 1024), F8)
    d_b1 = nc.dram_tensor("b1t", (128, 4), F32, kind="ExternalInput")
    d_cU = nc.dram_tensor("cUt", (128, 4), F32, kind="ExternalInput")
    d_b4 = nc.dram_tensor("b4t", (128, 4), F32, kind="ExternalInput")
    d_out = nc.dram_tensor("out", (BS, MODES), F16, kind="ExternalOutput")

    with tile.TileContext(nc) as tc:
        with tc.tile_pool(name="cpool", bufs=1) as cpool, \
             tc.tile_pool(name="wp", bufs=8) as wp, \
             tc.tile_pool(name="wp5", bufs=6) as wp5, \
             tc.tile_pool(name="sp5", bufs=8) as sp5:
            xpix = cpool.tile([128, XCOLS], F16, tag="xpix")
            hgT = cpool.tile([128, 128], F16, tag="hgT")
            b1t = cpool.tile([128, 4], F32, tag="b1t")
            cUs = cpool.tile([128, 4], F32, tag="cUs")
            b4s = cpool.tile([128, 4], F32, tag="b4s")
            uw = cpool.tile([128, 8, 512], F16, tag="uw")
            hT = cpool.tile([128, 4, 32], F16, tag="hT")
            h1T = cpool.tile([128, 4, 32], F16, tag="h1T")
            h2T = cpool.tile([128, 4, 32], F16, tag="h2T")

            # big stream on sync; small consts on gpsimd (SWDGE: no HWDGE
            # contention with the stream's descriptor generation)
            nc.sync.dma_start(xpix[:], d_xpix[:])
            nc.sync.dma_start(hgT[:], d_hgT[:])
            for t, dref in ((b1t, d_b1), (cUs, d_cU), (b4s, d_b4)):
                nc.gpsimd.dma_start(t[:], dref[:])

            vx = xpix.rearrange("p (b k) -> p b k", b=33)
            vh = hgT.rearrange("p (b k) -> p b k", b=32)

            # ======= L1: hT = relu(W^T [halo|x]^T + b1), computed directly
            # in transposed form: weights are the stationary operand (out ap
            # is only 32), so no transposes and a per-partition bias.
            with tc.tile_pool(name="ps1", bufs=1, space="PSUM") as ps1:
                accs = [ps1.tile([128, 32], F32, tag=f"acc{nb}",
                                 name=f"acc{nb}") for nb in range(4)]
                for K4 in range(25):
                    wt = wp.tile([128, 4, 512], F8, tag="wt")
                    nc.sync.dma_start(wt[:], d_w1f[K4])
                    for jj in range(4):
                        q = 4 * K4 + jj
                        src = vh[:, :, q] if q < 4 else vx[:, 0:32, q - 4]
                        for nb in range(4):
                            nc.tensor.matmul(
                                accs[nb][:],
                                wt[:, jj, nb * 128:(nb + 1) * 128], src,
                                start=(q == 0), stop=(q == 99))
                # uw rides the stream right after w1f (inside the pool scope
                # so no released-zone barrier blocks it)
                nc.sync.dma_start(uw[:], d_uw[:])
                for nb in range(4):
                    nc.scalar.activation(hT[:, nb, :], accs[nb][:], AF.Relu,
                                         bias=b1t[:, nb:nb + 1])

            # ======= L2 (U) and L4 (W4): weights-stationary 512->512 ========
            # h2's fb=3 block carries the 2^-K5 compensation for the fp8
            # quarter of W5f (relu commutes; b4t col 3 is pre-scaled).
            with tc.tile_pool(name="ps2", bufs=4, space="PSUM") as ps2:
                for wo, bias, src, dst in ((0, cUs, hT, h1T),
                                           (4, b4s, h1T, h2T)):
                    for f2b in range(4):
                        acc2 = ps2.tile([128, 32], F32, tag="acc2")
                        for fb in range(4):
                            nc.tensor.matmul(
                                acc2[:],
                                uw[:, wo + f2b, fb * 128:(fb + 1) * 128],
                                src[:, fb, :], start=(fb == 0), stop=(fb == 3))
                        scl = (2.0 ** (-K5)) if (wo == 4 and f2b == 3) else 1.0
                        nc.scalar.activation(dst[:, f2b, :], acc2[:], AF.Relu,
                                             bias=bias[:, f2b:f2b + 1],
                                             scale=scl)

            # ======= L5: out = h2 @ W5f  (bias added on host) ===============
            with tc.tile_pool(name="ps5", bufs=6, space="PSUM") as ps5, \
                 tc.tile_pool(name="psq", bufs=2, space="PSUM") as psq:
                for mc2 in range(12):
                    wt16 = wp5.tile([128, 3, 1024], F16, tag="w5a")
                    wt8 = wp5.tile([128, 1024], F8, tag="w5b")
                    if mc2 < 11:
                        nc.sync.dma_start(wt16[:], d_w5f16[mc2])
                        nc.sync.dma_start(wt8[:], d_w5f8[mc2])
                    else:
                        nc.sync.dma_start(wt16[:, :, 0:512],
                                          d_w5f16[mc2][:, :, 0:512])
                        nc.sync.dma_start(wt8[:], d_w5f8[mc2])
                        nc.sync.dma_start(wt16[:, :, 512:768],
                                          d_w5f16[mc2][:, :, 512:768])
                        nc.sync.dma_start(wt16[:, :, 768:1024],
                                          d_w5f16[mc2][:, :, 768:1024])
                    if mc2 < 11:
                        for half in range(2):
                            acc5 = ps5.tile([32, 512], F32, tag="acc5")
                            for fb in range(3):
                                nc.tensor.matmul(
                                    acc5[:], h2T[:, fb, :],
                                    wt16[:, fb, half * 512:(half + 1) * 512],
                                    start=(fb == 0), stop=False)
                            nc.tensor.matmul(
                                acc5[:], h2T[:, 3, :],
                                wt8[:, half * 512:(half + 1) * 512],
                                start=False, stop=True)
                            if half == 0:
                                osb = sp5.tile([32, 1024], F16, tag="osb")
                                nc.scalar.copy(osb[:, 0:512], acc5[:])
                            else:
                                nc.vector.tensor_copy(osb[:, 512:1024],
                                                      acc5[:])
                                nc.gpsimd.dma_start(
                                    d_out[:, mc2 * 1024:(mc2 + 1) * 1024],
                                    osb[:])
                    else:
                        # final group: chunk 22 as usual; chunk 23 in two
                        # 256-col sub-accs with the (early-arriving) fp8 fb=3
                        # matmul first, so only three ap-256 matmuls, a small
                        # copy and a small sync-queue writeback trail the last
                        # stream byte.
                        acc5 = ps5.tile([32, 512], F32, tag="acc5")
                        for fb in range(3):
                            nc.tensor.matmul(acc5[:], h2T[:, fb, :],
                                             wt16[:, fb, 0:512],
                                             start=(fb == 0), stop=False)
                        nc.tensor.matmul(acc5[:], h2T[:, 3, :], wt8[:, 0:512],
                                         start=False, stop=True)
                        osb = sp5.tile([32, 1024], F16, tag="osb")
                        nc.scalar.copy(osb[:, 0:512], acc5[:])
                        for sub in range(2):
                            c0 = 512 + sub * 256
                            accq = psq.tile([32, 256], F32, tag="accq")
                            nc.tensor.matmul(accq[:], h2T[:, 3, :],
                                             wt8[:, c0:c0 + 256],
                                             start=True, stop=False)
                            for fb in range(3):
                                nc.tensor.matmul(
                                    accq[:], h2T[:, fb, :],
                                    wt16[:, fb, c0:c0 + 256],
                                    start=False, stop=(fb == 2))
                            if sub == 0:
                                nc.vector.tensor_copy(osb[:, 512:768],
                                                      accq[:])
                                nc.gpsimd.dma_start(
                                    d_out[:, mc2 * 1024:mc2 * 1024 + 768],
                                    osb[:, 0:768])
                            else:
                                nc.vector.tensor_copy(osb[:, 768:1024],
                                                      accq[:])
                                nc.sync.dma_start(
                                    d_out[:, mc2 * 1024 + 768:
                                          (mc2 + 1) * 1024],
                                    osb[:, 768:1024])

    nc.compile()
    return nc


def _get_nc():
    global _NC_CACHE
    if _NC_CACHE is None:
        _NC_CACHE = _build_nc()
    return _NC_CACHE


def _make_in_maps(x, Ws1, bs1, Ws2, bs2, Wn1, bn1, Wn2, bn2, Wn3, bn3):
    shared = _fold_weights(Ws1, bs1, Ws2, bs2, Wn1, bn1, Wn2, bn2, Wn3, bn3)
    xscale = shared.pop("_xscale")
    b5f = shared.pop("_b5f")

    # halo: last 512 DFT-real values of every channel-2 image
    hg_all = np.real(np.fft.fft2(x[:, 2]))[:, 56:64, :].reshape(B, 512)
    hg_all = (hg_all.reshape(B, 4, 128)
              * xscale[0:4][None, :, None]).astype(np.float16)

    in_maps = []
    for g in range(NCORE):
        xc = (x[g * BS:(g + 1) * BS].reshape(BS, 96, 128)
              * xscale[None, 4:, None]).astype(np.float16)
        xpix = np.zeros((128, XCOLS), np.float16)
        xpix[:, :BS * 96] = xc.reshape(BS * 96, 128).T
        hgT = np.zeros((128, 128), np.float16)
        for b in range(BS):
            gi = g * BS + b - 1
            if gi >= 0:
                hgT[:, 4 * b:4 * b + 4] = hg_all[gi].T
        in_maps.append({"xpix": np.ascontiguousarray(xpix),
                        "hgT": np.ascontiguousarray(hgT), **shared})
    return in_maps, b5f


def kernel(**inputs):
    x = np.ascontiguousarray(inputs["x"], dtype=np.float32)
    nc = _get_nc()
    in_maps, b5f = _make_in_maps(
        x, inputs["Ws1"], inputs["bs1"], inputs["Ws2"], inputs["bs2"],
        inputs["Wn1"], inputs["bn1"], inputs["Wn2"], inputs["bn2"],
        inputs["Wn3"], inputs["bn3"])
    res = run_bass_kernel_spmd(nc, in_maps, list(range(NCORE)))
    out = np.empty((B, C, H, W), np.float32)
    for g in range(NCORE):
        out[g * BS:(g + 1) * BS] = (
            (res.results[g]["out"].astype(np.float32) + b5f)
            .reshape(BS, C, H, W))
    return out


# revision 9
# speedup vs baseline: 3.6728x; 1.0737x over previous
"""Trainium2 Bass kernel for nn_EnhancedFractionalPINO.

Math folding (all precomputed on host, per call):
  reference out = iDFT( relu(relu(relu(GLconv(DFT(x))@Ws1+b1) @ (Ws2@Wn1)
                  + (bs2@Wn1+bn1)) @ Wn2 + bn2) @ Wn3 + bn3 )
  - Ws2@Wn1 folds to a single 512x512 matrix U (no relu between the two
    12288-wide matmuls in the reference), eliminating both of them.
  - The GL fractional conv (lower-triangular Toeplitz T0 within a batch row
    plus a 512-sample halo from the previous batch) and the forward 2-D DFT
    fold into Ws1:  V0_pix = D^T T0^T Ws1 acts on raw pixels;  a 512x512
    V1h acts on the last 512 DFT values of the previous batch's channel-2
    image (computed on host via fft2).  Full in-batch GL taps.
  - The inverse 2-D DFT folds into Wn3: W5f = Wn3 o blockdiag(Re(iDFT)).
  - The final bias (b5f = bn3 o iDFT) is added on the host (linear).

Precision: L1 weights are float8-e3m4 with a per-chunk pow2 scale compensated
exactly in the disjoint x / halo column groups; the last K-quarter of W5f is
e3m4 with a pow2 scale compensated in h2's fb=3 block (relu commutes with
positive scales).  All other tensors fp16; PSUM accumulation fp32.

Per core (batch-parallel, 32 batches/core): a 4-layer MLP
  h   = relu([halo | x_pixels] @ [V1h; V0_pix] + b1)      (K=12800 streamed)
  h1  = relu(h @ U + cU);  h2 = relu(h1 @ W4 + b4)        (weights resident)
  out = h2 @ W5f                                          (N=12288 streamed)
"""

import numpy as np

import concourse.bass as bass
import concourse.mybir as mybir
import concourse.tile as tile
from concourse import bacc
from concourse.bass_utils import run_bass_kernel_spmd

F32 = mybir.dt.float32
F16 = mybir.dt.float16
F8 = mybir.dt.float8e3
AF = mybir.ActivationFunctionType

B, C, H, W = 256, 3, 64, 64
MODES = C * H * W              # 12288
NTOT = B * MODES
ALPHA = 0.5
NCORE = 8
BS = B // NCORE                # 32 batches per core
XCOLS = 33 * 96 + 128          # 3072 pixel chunks + view pad + 128 halo cols

LAM_H, LAM_1, LAM_2 = 16.0, 4.0, 4.0
K5 = 11                        # pow2 scale exponent for the fp8 block of W5f


# ---------------------------------------------------------------- host folds
def _fold_weights(Ws1, bs1, Ws2, bs2, Wn1, bn1, Wn2, bn2, Wn3, bn3):
    f16 = lambda a: np.ascontiguousarray(a, dtype=np.float16)
    f83 = mybir.dt.np(F8)
    s = float(np.float64(1.0 / (NTOT - 1)) ** (-ALPHA))

    # GL weights w_j (enough taps for in-batch + 512-halo reach)
    j = np.arange(1, 13312, dtype=np.float64)
    wgl = np.concatenate([[1.0], np.cumprod((j - 1.0 - ALPHA) / j)])

    # V0[m] = sum_d w_d W1s[m+d];  V1h[m'] = sum_k w_{k+512-m'} W1s[k]
    L = 32768
    W1s = Ws1.astype(np.float64) * (s / LAM_H)
    corr = np.fft.irfft(
        np.fft.rfft(W1s, n=L, axis=0) * np.conj(np.fft.rfft(wgl, n=L))[:, None],
        n=L, axis=0)
    V0 = corr[:MODES].astype(np.float32)
    V1h = corr[L - 512:].astype(np.float32)

    jk = np.outer(np.arange(64), np.arange(64)).astype(np.float64)
    Cm = np.cos(2 * np.pi * jk / 64).astype(np.float32)
    Sm = np.sin(2 * np.pi * jk / 64).astype(np.float32)

    # V0_pix[(y,z),n] = sum_{u,v} (C[u,y]C[v,z] - S[u,y]S[v,z]) V0[(u,v),n]
    V0c = V0.reshape(3, 64, 64, 512)
    V0_pix = (np.einsum('uy,cuvn,vz->cyzn', Cm, V0c, Cm, optimize=True)
              - np.einsum('uy,cuvn,vz->cyzn', Sm, V0c, Sm, optimize=True)
              ).reshape(MODES, 512)
    Vcat = np.concatenate([V1h, V0_pix], axis=0)            # (12800, 512)

    # e3m4 per-chunk pow2 scaling; the scale is compensated exactly in the
    # (disjoint) x / halo column groups.
    am = np.abs(Vcat.reshape(100, 128 * 512)).max(axis=1)
    kq = np.clip(np.floor(np.log2(15.5 / np.maximum(am, 1e-12))), -12, 12)
    Vq8 = (Vcat.reshape(100, 128, 512)
           * (2.0 ** kq)[:, None, None].astype(np.float32)).astype(f83)
    xscale = (2.0 ** (-kq)).astype(np.float32)

    U = (Ws2.astype(np.float32) @ Wn1.astype(np.float32)) * np.float32(LAM_H / LAM_1)
    cU = ((bs2.astype(np.float32) @ Wn1.astype(np.float32) + bn1)
          / np.float32(LAM_1))
    W4 = Wn2 * np.float32(LAM_1 / LAM_2)

    # W5f = (Wn3 o Re(iDFT)) * LAM_2 ; b5f = bn3 o Re(iDFT)  (host-added)
    W5c = Wn3.astype(np.float32).reshape(512, 3, 64, 64)
    W5f = ((np.einsum('rcuv,uy,vz->rcyz', W5c, Cm, Cm, optimize=True)
            - np.einsum('rcuv,uy,vz->rcyz', W5c, Sm, Sm, optimize=True))
           * np.float32(LAM_2 / 4096.0)).reshape(512, MODES)
    b5c = bn3.astype(np.float32).reshape(3, 64, 64)
    b5f = ((np.einsum('cuv,uy,vz->cyz', b5c, Cm, Cm, optimize=True)
            - np.einsum('cuv,uy,vz->cyz', b5c, Sm, Sm, optimize=True))
           / np.float32(4096.0)).reshape(MODES)

    # split W5f: K-blocks fb=0..2 fp16, fb=3 e3m4 scaled by 2^K5 (compensated
    # via h2's fb=3 block, scaled 2^-K5 at the L4 activation)
    W5r = W5f.reshape(4, 128, 12, 1024)
    lim = np.float32(15.5 * 2.0 ** (-K5))
    W5r = np.concatenate([W5r[0:2], np.clip(W5r[2:4], -lim, lim)])
    b4t = (bn2 / LAM_2).reshape(4, 128).T.astype(np.float32).copy()
    b4t[:, 2:4] *= np.float32(2.0 ** (-K5))

    return {
        "w1f": np.ascontiguousarray(
            Vq8.reshape(25, 4, 128, 512).transpose(0, 2, 1, 3)),
        "_xscale": xscale,
        "_b5f": b5f,
        "uw": f16(np.concatenate(
            [U.reshape(4, 128, 4, 128).transpose(2, 1, 0, 3)
              .reshape(4, 128, 512).transpose(1, 0, 2),
             W4.reshape(4, 128, 4, 128).transpose(2, 1, 0, 3)
              .reshape(4, 128, 512).transpose(1, 0, 2)], axis=1)),
        "w5f16": f16(W5r[0:2].transpose(2, 1, 0, 3)),
        "w5f8": np.ascontiguousarray(
            (W5r[2:4] * np.float32(2.0 ** K5)).transpose(2, 1, 0, 3)
            .astype(f83)),
        "b1t": np.ascontiguousarray((bs1 / LAM_H).reshape(4, 128).T,
                                    dtype=np.float32),
        "cUt": np.ascontiguousarray(cU.reshape(4, 128).T, dtype=np.float32),
        "b4t": np.ascontiguousarray(b4t),
        "idn32": f16(np.eye(32)),
        "ones1": f16(np.ones((1, 32))),
    }


# ---------------------------------------------------------------- bass module
_NC_CACHE = None


def _build_nc():
    nc = bacc.Bacc("TRN2", target_bir_lowering=False, debug=False,
                   num_devices=NCORE)

    def din(name, shape, dt=F16):
        return nc.dram_tensor(name, shape, dt, kind="ExternalInput")

    d_xpix = din("xpix", (128, XCOLS))
    d_w1f = din("w1f", (25, 128, 4, 512), F8)
    d_uw = din("uw", (128, 8, 512))
    d_w5f16 = din("w5f16", (12, 128, 2, 1024))
    d_w5f8 = din("w5f8", (12, 128, 2, 1024), F8)
    d_b1 = nc.dram_tensor("b1t", (128, 4), F32, kind="ExternalInput")
    d_cU = nc.dram_tensor("cUt", (128, 4), F32, kind="ExternalInput")
    d_b4 = nc.dram_tensor("b4t", (128, 4), F32, kind="ExternalInput")
    d_out = nc.dram_tensor("out", (BS, MODES), F16, kind="ExternalOutput")

    with tile.TileContext(nc) as tc:
        with tc.tile_pool(name="cpool", bufs=1) as cpool, \
             tc.tile_pool(name="wp", bufs=8) as wp, \
             tc.tile_pool(name="wp5", bufs=6) as wp5, \
             tc.tile_pool(name="sp5", bufs=8) as sp5:
            xpix = cpool.tile([128, XCOLS], F16, tag="xpix")
            b1t = cpool.tile([128, 4], F32, tag="b1t")
            cUs = cpool.tile([128, 4], F32, tag="cUs")
            b4s = cpool.tile([128, 4], F32, tag="b4s")
            uw = cpool.tile([128, 8, 512], F16, tag="uw")
            hT = cpool.tile([128, 4, 32], F16, tag="hT")
            h1T = cpool.tile([128, 4, 32], F16, tag="h1T")
            h2T = cpool.tile([128, 4, 32], F16, tag="h2T")

            # big stream on sync; small consts on gpsimd (SWDGE: no HWDGE
            # contention with the stream's descriptor generation)
            nc.sync.dma_start(xpix[:], d_xpix[:])
            for t, dref in ((b1t, d_b1), (cUs, d_cU), (b4s, d_b4)):
                nc.scalar.dma_start(t[:], dref[:])

            vx = xpix[:, 0:3168].rearrange("p (b k) -> p b k", b=33)
            vh = xpix[:, 3168:3296].rearrange("p (b k) -> p b k", b=32)

            # ======= L1: hT = relu(W^T [halo|x]^T + b1), computed directly
            # in transposed form: weights are the stationary operand (out ap
            # is only 32), so no transposes and a per-partition bias.
            with tc.tile_pool(name="ps1", bufs=1, space="PSUM") as ps1:
                accs = [ps1.tile([128, 32], F32, tag=f"acc{nb}",
                                 name=f"acc{nb}") for nb in range(4)]
                for K4 in range(25):
                    wt = wp.tile([128, 4, 512], F8, tag="wt")
                    nc.sync.dma_start(wt[:], d_w1f[K4])
                    for jj in range(4):
                        q = 4 * K4 + jj
                        src = vh[:, :, q] if q < 4 else vx[:, 0:32, q - 4]
                        for nb in range(4):
                            nc.tensor.matmul(
                                accs[nb][:],
                                wt[:, jj, nb * 128:(nb + 1) * 128], src,
                                start=(q == 0), stop=(q == 99))
                # uw rides the stream right after w1f (inside the pool scope
                # so no released-zone barrier blocks it)
                nc.sync.dma_start(uw[:], d_uw[:])
                for nb in range(4):
                    nc.scalar.activation(hT[:, nb, :], accs[nb][:], AF.Relu,
                                         bias=b1t[:, nb:nb + 1])

            # ======= L2 (U) and L4 (W4): weights-stationary 512->512 ========
            # h2's fb=3 block carries the 2^-K5 compensation for the fp8
            # quarter of W5f (relu commutes; b4t col 3 is pre-scaled).
            with tc.tile_pool(name="ps2", bufs=4, space="PSUM") as ps2:
                for wo, bias, src, dst in ((0, cUs, hT, h1T),
                                           (4, b4s, h1T, h2T)):
                    for f2b in range(4):
                        acc2 = ps2.tile([128, 32], F32, tag="acc2")
                        for fb in range(4):
                            nc.tensor.matmul(
                                acc2[:],
                                uw[:, wo + f2b, fb * 128:(fb + 1) * 128],
                                src[:, fb, :], start=(fb == 0), stop=(fb == 3))
                        scl = (2.0 ** (-K5)) if (wo == 4 and f2b >= 2) else 1.0
                        nc.scalar.activation(dst[:, f2b, :], acc2[:], AF.Relu,
                                             bias=bias[:, f2b:f2b + 1],
                                             scale=scl)

            # ======= L5: out = h2 @ W5f  (bias added on host) ===============
            with tc.tile_pool(name="ps5", bufs=6, space="PSUM") as ps5, \
                 tc.tile_pool(name="psq", bufs=2, space="PSUM") as psq:
                for mc2 in range(12):
                    wt16 = wp5.tile([128, 2, 1024], F16, tag="w5a")
                    wt8 = wp5.tile([128, 2, 1024], F8, tag="w5b")
                    if mc2 < 11:
                        nc.sync.dma_start(wt16[:], d_w5f16[mc2])
                        nc.sync.dma_start(wt8[:], d_w5f8[mc2])
                    else:
                        nc.sync.dma_start(wt16[:, :, 0:512],
                                          d_w5f16[mc2][:, :, 0:512])
                        nc.sync.dma_start(wt8[:], d_w5f8[mc2])
                        nc.sync.dma_start(wt16[:, :, 512:768],
                                          d_w5f16[mc2][:, :, 512:768])
                        nc.sync.dma_start(wt16[:, :, 768:1024],
                                          d_w5f16[mc2][:, :, 768:1024])
                    if mc2 < 11:
                        for half in range(2):
                            acc5 = ps5.tile([32, 512], F32, tag="acc5")
                            sl = slice(half * 512, (half + 1) * 512)
                            for fb in range(2):
                                nc.tensor.matmul(
                                    acc5[:], h2T[:, fb, :], wt16[:, fb, sl],
                                    start=(fb == 0), stop=False)
                            for fb in range(2):
                                nc.tensor.matmul(
                                    acc5[:], h2T[:, 2 + fb, :],
                                    wt8[:, fb, sl],
                                    start=False, stop=(fb == 1))
                            if half == 0:
                                osb = sp5.tile([32, 1024], F16, tag="osb")
                                nc.scalar.copy(osb[:, 0:512], acc5[:])
                            else:
                                nc.vector.tensor_copy(osb[:, 512:1024],
                                                      acc5[:])
                                nc.gpsimd.dma_start(
                                    d_out[:, mc2 * 1024:(mc2 + 1) * 1024],
                                    osb[:])
                    else:
                        # final group: chunk 22 as usual; chunk 23 in two
                        # 256-col sub-accs with the (early-arriving) fp8
                        # matmuls first, so only two ap-256 matmuls, a small
                        # copy and a small sync-queue writeback trail the last
                        # stream byte.
                        acc5 = ps5.tile([32, 512], F32, tag="acc5")
                        for fb in range(2):
                            nc.tensor.matmul(acc5[:], h2T[:, fb, :],
                                             wt16[:, fb, 0:512],
                                             start=(fb == 0), stop=False)
                        for fb in range(2):
                            nc.tensor.matmul(acc5[:], h2T[:, 2 + fb, :],
                                             wt8[:, fb, 0:512],
                                             start=False, stop=(fb == 1))
                        osb = sp5.tile([32, 1024], F16, tag="osb")
                        nc.scalar.copy(osb[:, 0:512], acc5[:])
                        for sub in range(2):
                            c0 = 512 + sub * 256
                            sq = slice(c0, c0 + 256)
                            accq = psq.tile([32, 256], F32, tag="accq")
                            for fb in range(2):
                                nc.tensor.matmul(accq[:], h2T[:, 2 + fb, :],
                                                 wt8[:, fb, sq],
                                                 start=(fb == 0), stop=False)
                            for fb in range(2):
                                nc.tensor.matmul(
                                    accq[:], h2T[:, fb, :],
                                    wt16[:, fb, sq],
                                    start=False, stop=(fb == 1))
                            if sub == 0:
                                nc.vector.tensor_copy(osb[:, 512:768],
                                                      accq[:])
                                nc.gpsimd.dma_start(
                                    d_out[:, mc2 * 1024:mc2 * 1024 + 768],
                                    osb[:, 0:768])
                            else:
                                nc.vector.tensor_copy(osb[:, 768:1024],
                                                      accq[:])
                                nc.sync.dma_start(
                                    d_out[:, mc2 * 1024 + 768:
                                          (mc2 + 1) * 1024],
                                    osb[:, 768:1024])

    nc.compile()
    return nc


def _get_nc():
    global _NC_CACHE
    if _NC_CACHE is None:
        _NC_CACHE = _build_nc()
    return _NC_CACHE


def _make_in_maps(x, Ws1, bs1, Ws2, bs2, Wn1, bn1, Wn2, bn2, Wn3, bn3):
    shared = _fold_weights(Ws1, bs1, Ws2, bs2, Wn1, bn1, Wn2, bn2, Wn3, bn3)
    xscale = shared.pop("_xscale")
    b5f = shared.pop("_b5f")

    # halo: last 512 DFT-real values of every channel-2 image
    hg_all = np.real(np.fft.fft2(x[:, 2]))[:, 56:64, :].reshape(B, 512)
    hg_all = (hg_all.reshape(B, 4, 128)
              * xscale[0:4][None, :, None]).astype(np.float16)

    in_maps = []
    for g in range(NCORE):
        xc = (x[g * BS:(g + 1) * BS].reshape(BS, 96, 128)
              * xscale[None, 4:, None]).astype(np.float16)
        xpix = np.zeros((128, XCOLS), np.float16)
        xpix[:, :BS * 96] = xc.reshape(BS * 96, 128).T
        for b in range(BS):
            gi = g * BS + b - 1
            if gi >= 0:
                xpix[:, 3168 + 4 * b:3168 + 4 * b + 4] = hg_all[gi].T
        in_maps.append({"xpix": np.ascontiguousarray(xpix), **shared})
    return in_maps, b5f


def kernel(**inputs):
    x = np.ascontiguousarray(inputs["x"], dtype=np.float32)
    nc = _get_nc()
    in_maps, b5f = _make_in_maps(
        x, inputs["Ws1"], inputs["bs1"], inputs["Ws2"], inputs["bs2"],
        inputs["Wn1"], inputs["bn1"], inputs["Wn2"], inputs["bn2"],
        inputs["Wn3"], inputs["bn3"])
    res = run_bass_kernel_spmd(nc, in_maps, list(range(NCORE)))
    out = np.empty((B, C, H, W), np.float32)
    for g in range(NCORE):
        out[g * BS:(g + 1) * BS] = (
            (res.results[g]["out"].astype(np.float32) + b5f)
            .reshape(BS, C, H, W))
    return out
